# revision 13
# baseline (speedup 1.0000x reference)
"""Trainium2 Bass kernel for a dense transformer block (pre-LN attn + MLP).

B=4, T=2048, D=768, H=12 (DH=64), DFF=3072, fp32.

Sharding: 8 cores = 4 batches x 2 roles. Each core processes one batch and
owns 1024 query tokens (two 512-blocks, paired {0,3}/{1,2} for causal load
balance). K/V are computed for the full 2048 tokens on both cores of a batch
(cheap), so there are NO collectives.

SPMD uniformity: all 8 cores run ONE identical NEFF. Causal structure is
carried in DATA, not code:
  - host permutes each batch's token axis to [own0, own1, otherA, otherB]
  - q-slot0 attends s-chunks {0..3, 8..11}; q-slot1 attends s-chunks {0..15}
  - per-(slot,chunk) exp scale/bias inputs select live / dead (zero) chunks
  - 4 canonical triangular masks handle the self-diagonal 512-blocks

Everything on-chip runs in a transposed layout (features on partitions,
tokens on the free axis). Matmul cost on the PE depends only on the output
free size, so per-token LayerNorm statistics are computed REPLICATED across
all 128 partitions (ones [128,128] lhsT) and softmax denominators come
replicated across 64 partitions from a shared ones-block appended to V —
no partition broadcasts are needed anywhere.

Engine assignment: PE does GEMMs + LN column-sum stats; Act does exp and the
LN apply (scale/bias copy); DVE does LN tensor ops, masks, softmax divides
and residuals; Pool (gpsimd) does all PSUM->SBUF copies, squares and relu.
Emission is software-pipelined: slot0 attention is woven with the remaining
QKV projections, slot1 attention with the first half of the MLP, so the PE
never idles behind the Act exp chain.
"""

import sys

sys.path.insert(0, "/opt/trn_rl_repo")

from contextlib import ExitStack

import numpy as np

import concourse.bass as bass
import concourse.mybir as mybir
import concourse.tile as tile
from concourse import bacc
from concourse.bass_utils import run_bass_kernel_spmd

F32 = mybir.dt.float32
F32R = mybir.dt.float32r
AF = mybir.ActivationFunctionType
BF16 = mybir.dt.bfloat16
ALU = mybir.AluOpType

H, D, DFF = 12, 768, 3072
DH = 64
B, T = 4, 2048
EPS = 1e-5
P = 128
NC = D // P          # 6 feature chunks
NF = DFF // P        # 24 ff tiles
TB = 512             # token block
NTB = T // TB        # 4 blocks
VW = H * 65          # 780: per head 64 values + 1 ones col
SLOT_CHUNKS = [[0, 1, 2, 3, 8, 9, 10, 11], list(range(16))]
# role -> permuted block order [own0, own1, restA, restB] (original block ids)
ROLE_ORDER = [[0, 3, 1, 2], [1, 2, 0, 3]]
DEAD = -30000.0      # exp(DEAD) == 0 in fp32
TBORD = [0, 2, 1, 3]  # emission order: slot0 needs permuted blocks 0 and 2

_cached = {}
PHASE_MARKS = []


def _mark(nc, name):
    PHASE_MARKS.append((name, nc.next_id()))


def _weave(primary, filler, fill_every=1, fill_n=1):
    """Drain primary generator; after every `fill_every` primary units, pull
    `fill_n` units from filler. Any leftover filler drains at the end."""
    i = 0
    f = iter(filler)
    for _ in primary:
        i += 1
        if i % fill_every == 0:
            for _ in range(fill_n):
                if next(f, StopIteration) is StopIteration:
                    break
    for _ in f:
        pass


def _build_nc():
    nc = bacc.Bacc("TRN2", target_bir_lowering=False, debug=False,
                   enable_asserts=False, num_devices=8)

    def din(name, shape, dt=F32R):
        return nc.dram_tensor(name, shape, dt, kind="ExternalInput").ap()

    xt_d = din("xt", [D, T])                 # X[b].T, token-permuted
    wqt_d = din("wqt", [D, D], BF16)         # w_q as [c, m]
    wkt_d = din("wkt", [D, D], BF16)
    wvt_d = din("wvt", [D, D], BF16)
    wo_d = din("wo", [D, D], BF16)           # natural [m, c]
    w1t_d = din("w1t", [D, DFF], BF16)       # W1.T  [c, f]
    w2t_d = din("w2t", [DFF, D], BF16)       # W2.T  [f, c]
    onesbc_d = din("onesbc", [P, P])         # all-ones lhsT: replicated sums
    masks_d = din("masks", [4, P, TB], BF16)  # tri masks
    scalein_d = din("scalein", [P, 24], F32) # exp scale per (slot,chunk)
    biasin_d = din("biasin", [P, 24], F32)   # exp bias per (slot,chunk)
    g1_d = din("g1v", [D], F32)
    be1_d = din("be1v", [D], F32)
    g2_d = din("g2v", [D], F32)
    be2_d = din("be2v", [D], F32)
    b1_d = din("b1v", [DFF], F32)
    b2_d = din("b2v", [D], F32)

    outt_d = nc.dram_tensor("outt", [D, 1024], F32, kind="ExternalOutput").ap()

    xt_r = xt_d.rearrange("(j p) t -> p j t", p=P)

    with tile.TileContext(nc) as tc, ExitStack() as ctx, \
         nc.allow_low_precision(reason="fp32r/bf16 intermediates are intended"):
        consts = ctx.enter_context(tc.tile_pool(name="consts", bufs=1))
        ps = ctx.enter_context(tc.tile_pool(name="ps", bufs=1, space="PSUM"))
        work = ctx.enter_context(tc.tile_pool(name="work", bufs=2))
        p_xp = ctx.enter_context(tc.tile_pool(name="p_xp", bufs=1))
        xp_sb = p_xp.tile([P, NC, 1024], F32R, tag="xp", name="xp")
        p_yt = ctx.enter_context(tc.tile_pool(name="p_yt", bufs=1))
        yt_all = p_yt.tile([P, NC, TB], BF16, tag="yt_all", name="yt_all")
        S = {"xp_sb": xp_sb}

        onesbc_sb = consts.tile([P, P], F32R, tag="onesbc")
        scale_sb = consts.tile([P, 24], F32, tag="scalein")
        bias_sb = consts.tile([P, 24], F32, tag="biasin")
        g1_sb = consts.tile([P, NC], F32, tag="g1")
        be1_sb = consts.tile([P, NC], F32, tag="be1")
        g2_sb = consts.tile([P, NC], F32, tag="g2")
        be2_sb = consts.tile([P, NC], F32, tag="be2")
        b1_sb = consts.tile([P, NF], F32, tag="b1")
        b2_sb = consts.tile([P, NC], F32, tag="b2")

        def _early_const_dmas():
            nc.sync.dma_start(onesbc_sb[:], onesbc_d)
            for sb, d in ((g1_sb, g1_d), (be1_sb, be1_d)):
                nc.sync.dma_start(sb[:], d.rearrange("(j p) -> p j", p=P))

        def _late_const_dmas():
            nc.sync.dma_start(scale_sb[:], scalein_d)
            nc.sync.dma_start(bias_sb[:], biasin_d)
            for sb, d in ((g2_sb, g2_d), (be2_sb, be2_d)):
                nc.sync.dma_start(sb[:], d.rearrange("(j p) -> p j", p=P))
            nc.sync.dma_start(b1_sb[:], b1_d.rearrange("(j p) -> p j", p=P))
            nc.sync.dma_start(b2_sb[:], b2_d.rearrange("(j p) -> p j", p=P))
            nc.sync.dma_start(masks_sb[:], masks_d.rearrange("o p t -> p o t"))

        # ---------------- LayerNorm helpers ----------------
        def ln_stats(src3):
            """src3: [128, NC, TB] slice. Returns (r, mur): [128, TB] tiles
            with 1/std and mu/std replicated across partitions."""
            s1 = ps.tile([P, TB], F32, tag="acc", bufs=2, name="s1")
            s2 = ps.tile([P, TB], F32, tag="acc", bufs=2, name="s2")
            for j in range(NC):
                nc.tensor.matmul(s1[:], onesbc_sb[:], src3[:, j, :],
                                 start=(j == 0), stop=(j == NC - 1))
            for j in range(NC):
                sq = work.tile([P, TB], F32R, tag="sq", bufs=1)
                nc.gpsimd.tensor_mul(sq[:], src3[:, j, :], src3[:, j, :])
                nc.tensor.matmul(s2[:], onesbc_sb[:], sq[:],
                                 start=(j == 0), stop=(j == NC - 1))
            mu = work.tile([P, TB], F32, tag="mu", bufs=1)
            t = work.tile([P, TB], F32, tag="tmp", bufs=1)
            r = work.tile([P, TB], F32R, tag="r", bufs=2)
            mur = work.tile([P, TB], F32R, tag="mur", bufs=2)
            nc.vector.tensor_scalar_mul(mu[:], s1[:], 1.0 / D)
            nc.vector.tensor_mul(t[:], mu[:], mu[:])
            nc.vector.scalar_tensor_tensor(t[:], s2[:], 1.0 / D, t[:],
                                           ALU.mult, ALU.subtract)
            nc.vector.tensor_scalar_add(t[:], t[:], EPS)
            nc.scalar.activation(t[:], t[:], AF.Sqrt)
            nc.vector.reciprocal(r[:], t[:])
            nc.vector.tensor_mul(mur[:], mu[:], r[:])
            return r, mur

        def ln_norm_chunk(src_j, dst_j, r, mur, g_sb, be_sb, j):
            """dst = (src*r - mur)*g[p] + be[p]."""
            t1 = work.tile([P, TB], F32R, tag="nrm1")
            nc.gpsimd.tensor_mul(t1[:], src_j, r[:])
            nc.gpsimd.tensor_sub(t1[:], t1[:], mur[:])
            nc.scalar.activation(dst_j, t1[:], AF.Identity,
                                 bias=be_sb[:, j:j + 1], scale=g_sb[:, j:j + 1])

        # ---------------- attention-phase SBUF tensors ----------------
        es_kqv = ExitStack()
        p_kqv = es_kqv.enter_context(tc.tile_pool(name="p_kqv", bufs=1,
                                                  side="right"))
        kt_sb = p_kqv.tile([P, NC, T], BF16, tag="kt")      # K^T [m, s]
        qt_sb = p_kqv.tile([P, NC, 1024], BF16, tag="qt")   # Q^T [m, t_own]
        v_sb = p_kqv.tile([P, 16, VW], BF16, tag="v")       # V_ext [s, 780]
        v_hv = v_sb.rearrange("p s (h e) -> p s h e", e=65)
        vd_view = v_hv[:, :, :, 0:64]
        nc.vector.memset(v_hv[:, :, :, 64:65], 1.0)

        es_masks = ExitStack()
        p_masks = es_masks.enter_context(tc.tile_pool(name="p_masks", bufs=1,
                                                      side="right"))
        p_e = es_masks.enter_context(tc.tile_pool(name="p_e", bufs=2,
                                                  side="right"))
        masks_sb = p_masks.tile([P, 4, TB], BF16, tag="masks")

        es_wqkv = ExitStack()
        p_wqkv = es_wqkv.enter_context(tc.tile_pool(name="p_wqkv", bufs=1,
                                                    side="right"))
        wq_sb = p_wqkv.tile([P, NC, D], BF16, tag="wq")
        wk_sb = p_wqkv.tile([P, NC, D], BF16, tag="wk")
        wv_sb = p_wqkv.tile([P, NC, D], BF16, tag="wv")

        # ---------------- Phase A-D: LN1 + QKV over TBORD, pipelined -------
        _mark(nc, "lnq")
        es_xn1 = ExitStack()
        p_xn1 = es_xn1.enter_context(tc.tile_pool(name="p_xn1", bufs=2))
        xn1_tiles = {}

        def norm_units(tb):
            r, mur = stats_res[tb]
            xn1_t = p_xn1.tile([P, NC, TB], BF16, tag="xn1", name=f"xn1_{tb}")
            xn1_tiles[tb] = xn1_t
            xt_t = xtr_tiles[tb]
            for j in range(NC):
                ln_norm_chunk(xt_t[:, j, :], xn1_t[:, j, :], r, mur,
                              g1_sb, be1_sb, j)
                yield

        def qkv_units(tb, want_q):
            tsl = slice(tb * TB, (tb + 1) * TB)
            xn1_t = xn1_tiles[tb]
            for mt in range(NC):
                msl = slice(mt * P, (mt + 1) * P)
                acc = ps.tile([P, TB], F32, tag="acc", bufs=2, name="ka")
                for j in range(NC):
                    nc.tensor.matmul(acc[:], wk_sb[:, j, msl], xn1_t[:, j, :],
                                     start=(j == 0), stop=(j == NC - 1))
                nc.vector.tensor_copy(kt_sb[:, mt, tsl], acc[:])
                yield
            for st in range(4):
                ssl = slice(st * P, (st + 1) * P)
                for half, fsl, w in ((0, slice(0, TB), TB),
                                     (1, slice(TB, D), D - TB)):
                    acc = ps.tile([P, TB], F32, tag="acc", bufs=2, name="va")
                    for j in range(NC):
                        nc.tensor.matmul(acc[:, :w], xn1_t[:, j, ssl],
                                         wv_sb[:, j, fsl],
                                         start=(j == 0), stop=(j == NC - 1))
                    src = acc[:, :w].rearrange("p (h e) -> p h e", e=64)
                    h0 = half * 8
                    nc.vector.tensor_copy(
                        vd_view[:, tb * 4 + st, h0:h0 + w // 64, :], src)
                    yield
            if want_q:
                for mt in range(NC):
                    msl = slice(mt * P, (mt + 1) * P)
                    acc = ps.tile([P, TB], F32, tag="acc", bufs=2, name="qa")
                    for j in range(NC):
                        nc.tensor.matmul(acc[:], wq_sb[:, j, msl],
                                         xn1_t[:, j, :],
                                         start=(j == 0), stop=(j == NC - 1))
                    nc.vector.tensor_copy(qt_sb[:, mt, tsl], acc[:])
                    yield

        es_xtr = ExitStack()
        p_xtr = es_xtr.enter_context(tc.tile_pool(name="p_xtr", bufs=2))
        xtr_tiles = {}
        stats_res = {}

        def load_xt(tb):
            tsl = slice(tb * TB, (tb + 1) * TB)
            xt_t = p_xtr.tile([P, NC, TB], F32R, tag="xtr", name=f"xt_{tb}")
            xtr_tiles[tb] = xt_t
            for j0 in range(0, NC, 2):
                nc.sync.dma_start(xt_t[:, j0:j0 + 2, :],
                                  xt_r[:, j0:j0 + 2, tsl])

        # A: tb0 stats
        load_xt(0)
        _early_const_dmas()
        nc.sync.dma_start(wk_sb[:], wkt_d.rearrange("(j p) m -> p j m", p=P))
        nc.sync.dma_start(wv_sb[:], wvt_d.rearrange("(j p) m -> p j m", p=P))
        nc.sync.dma_start(wq_sb[:], wqt_d.rearrange("(j p) m -> p j m", p=P))
        stats_res[0] = ln_stats(xtr_tiles[0][:])
        # B: tb2 stats; norm0 + qkv0
        load_xt(2)
        _late_const_dmas()
        stats_res[2] = ln_stats(xtr_tiles[2][:])
        for _ in norm_units(0):
            pass
        for _ in qkv_units(0, want_q=True):
            pass
        # C: tb1 stats; norm2 + qkv2
        load_xt(1)
        stats_res[1] = ln_stats(xtr_tiles[1][:])
        for _ in norm_units(2):
            pass
        for _ in qkv_units(2, want_q=False):
            pass
        # D: tb3 stats; norm1
        load_xt(3)
        stats_res[3] = ln_stats(xtr_tiles[3][:])
        for _ in norm_units(1):
            pass

        # ---------------- attention machinery ----------------

        def attn_units(sl_i):
            qsl = slice(sl_i * TB, (sl_i + 1) * TB)
            chunks = SLOT_CHUNKS[sl_i]
            last_ci = len(chunks) - 1
            for mt in range(NC):
                yt2 = [ps.tile([65, TB], F32, tag="yt", bufs=2,
                               name=f"yt{ph}") for ph in range(2)]

                def pv(ci, e_sb):
                    ch = chunks[ci]
                    for ph in range(2):
                        h = 2 * mt + ph
                        nc.tensor.matmul(
                            yt2[ph][:], v_sb[:, ch, h * 65:(h + 1) * 65],
                            e_sb[:, ph * TB:(ph + 1) * TB],
                            start=(ci == 0), stop=(ci == last_ci))

                pend = None
                for ci, ch in enumerate(chunks):
                    sb_idx = (0 if sl_i == 0 else 8) + ci
                    st2 = ps.tile([P, 2 * TB], F32, tag="st", bufs=2)
                    for ph in range(2):
                        o = ph * 64
                        nc.tensor.matmul(
                            st2[:, ph * TB:(ph + 1) * TB],
                            kt_sb[o:o + 64, mt, ch * P:(ch + 1) * P],
                            qt_sb[o:o + 64, mt, qsl], start=True, stop=True)
                    e_sb = p_e.tile([P, 2 * TB], BF16, tag="e")
                    nc.scalar.activation(
                        e_sb[:], st2[:], AF.Exp,
                        bias=bias_sb[:, sb_idx:sb_idx + 1],
                        scale=scale_sb[:, sb_idx:sb_idx + 1])
                    di = ch - 4 * sl_i
                    if 0 <= di < 4:
                        for ph in range(2):
                            nc.vector.tensor_mul(
                                e_sb[:, ph * TB:(ph + 1) * TB],
                                e_sb[:, ph * TB:(ph + 1) * TB],
                                masks_sb[:, di, :])
                    if pend is not None:
                        pv(*pend)
                    pend = (ci, e_sb)
                    yield
                pv(*pend)
                for ph in range(2):
                    o = ph * 64
                    rc = work.tile([1, TB], F32R, tag="rc")
                    nc.vector.reciprocal(rc[:], yt2[ph][64:65, :])
                    bc = ps.tile([64, TB], F32, tag="acc", bufs=2, name="bc")
                    nc.tensor.matmul(bc[:], onesbc_sb[0:1, 0:64], rc[:],
                                     start=True, stop=True)
                    nc.vector.tensor_copy(yt_all[o:o + 64, mt, :],
                                          yt2[ph][0:64, :])
                    nc.vector.tensor_mul(yt_all[o:o + 64, mt, :],
                                         yt_all[o:o + 64, mt, :], bc[:])
                yield

        def wo_ln2(sl_i):
            qsl = slice(sl_i * TB, (sl_i + 1) * TB)
            xn2_sb = S["xn2_sb"]
            for ct in range(NC):
                ao = ps.tile([P, TB], F32, tag="acc", bufs=2, name="ao")
                wo_sb = S["wo_sb"]
                for mc in range(NC):
                    nc.tensor.matmul(ao[:], wo_sb[:, mc, ct * P:(ct + 1) * P],
                                     yt_all[:, mc, :],
                                     start=(mc == 0), stop=(mc == NC - 1))
                nc.vector.tensor_add(xp_sb[:, ct, qsl],
                                     xp_sb[:, ct, qsl], ao[:])
            r2, mur2 = ln_stats(xp_sb[:, :, qsl])
            for j in range(NC):
                ln_norm_chunk(xp_sb[:, j, qsl], xn2_sb[:, j, qsl], r2, mur2,
                              g2_sb, be2_sb, j)

        # ---------------- MLP machinery ----------------
        w1t_r = w1t_d.rearrange("(j p) f -> p j f", p=P)
        w2t_r = w2t_d.rearrange("(f p) c -> p f c", p=P)
        outt_r = outt_d.rearrange("(j p) t -> p j t", p=P)

        def w1_units(sl_i):
            tsl = slice(sl_i * TB, (sl_i + 1) * TB)
            h1_sb, p_wmlp, xn2_sb = S["h1_sb"], S["p_wmlp"], S["xn2_sb"]
            for ft2 in range(NF // 2):
                w1_t = p_wmlp.tile([P, NC, 2 * P], BF16, tag="w1",
                                   name=f"w1_{sl_i}_{ft2}")
                nc.sync.dma_start(
                    w1_t[:], w1t_r[:, :, 2 * ft2 * P:(2 * ft2 + 2) * P])
                for sub in range(2):
                    ft = 2 * ft2 + sub
                    hp = ps.tile([P, TB], F32, tag="acc", bufs=2, name="hp")
                    for j in range(NC):
                        nc.tensor.matmul(hp[:],
                                         w1_t[:, j, sub * P:(sub + 1) * P],
                                         xn2_sb[:, j, tsl],
                                         start=(j == 0), stop=(j == NC - 1))
                    nc.vector.tensor_scalar(h1_sb[:, ft, tsl], hp[:],
                                            b1_sb[:, ft:ft + 1], 0.0,
                                            ALU.add, ALU.max)
                    yield

        def w2_units(g):
            h1_sb, p_wmlp = S["h1_sb"], S["p_wmlp"]
            p_out = S["p_out"]
            o2s = {}
            for ci, (t, bu) in enumerate((("acc", 2), ("yt", 2))):
                for tb in range(2):
                    o2s[(ci, tb)] = ps.tile([P, TB], F32, tag=t, bufs=bu,
                                            name=f"o2_{g}_{ci}_{tb}")
            stp = ps.tile([P, 2 * TB], F32, tag="st", bufs=2,
                          name=f"o2st_{g}")
            o2s[(2, 0)] = stp[:, 0:TB]
            o2s[(2, 1)] = stp[:, TB:2 * TB]
            for ft2 in range(NF // 2):
                w2_t = p_wmlp.tile([P, 2, D], BF16, tag="w2")
                nc.sync.dma_start(w2_t[:], w2t_r[:, 2 * ft2:2 * ft2 + 2, :])
                for sub in range(2):
                    ft = 2 * ft2 + sub
                    for ci in range(3):
                        ct = g * 3 + ci
                        for tb in range(2):
                            nc.tensor.matmul(
                                o2s[(ci, tb)][:],
                                w2_t[:, sub, ct * P:(ct + 1) * P],
                                h1_sb[:, ft, tb * TB:(tb + 1) * TB],
                                start=(ft == 0), stop=(ft == NF - 1))
                yield
            for tb in range(2):
                tsl = slice(tb * TB, (tb + 1) * TB)
                for ci in range(3):
                    ct = g * 3 + ci
                    ot = p_out.tile([P, TB], F32, tag="ot")
                    nc.vector.scalar_tensor_tensor(
                        ot[:], o2s[(ci, tb)][:], b2_sb[:, ct:ct + 1],
                        xp_sb[:, ct, tsl], ALU.add, ALU.add)
                    nc.sync.dma_start(outt_r[:, ct, tsl], ot[:])
                    yield

        # ---------------- Phase E: slot0 attention woven with rest --------
        _mark(nc, "attn0")

        def e_fill():
            for u in norm_units(3):
                yield u
            es_xtr.close()
            for u in qkv_units(1, want_q=True):
                yield u
            for u in qkv_units(3, want_q=False):
                yield u
            es_xn1.close()
            es_wqkv.close()
            p_h1 = ctx.enter_context(tc.tile_pool(name="p_h1", bufs=1))
            S["h1_sb"] = p_h1.tile([P, NF, 1024], BF16, tag="h1", name="h1")
            S["p_wmlp"] = ctx.enter_context(
                tc.tile_pool(name="p_wmlp", bufs=2))
            S["p_wo"] = es_wo = ExitStack()
            p_wo = es_wo.enter_context(tc.tile_pool(name="p_wo", bufs=1))
            S["wo_sb"] = wo_sb = p_wo.tile([P, NC, D], BF16, tag="wo", name="wo")
            nc.sync.dma_start(wo_sb[:],
                              wo_d.rearrange("(j p) m -> p j m", p=P))
            nc.sync.dma_start(xp_sb[:], xt_r[:, :, 0:1024])
            yield

        _weave(attn_units(0), e_fill(), fill_every=1, fill_n=1)

        # ---------------- Phase F: slot0 wo + LN2 ----------------
        _mark(nc, "wo0")
        S["p_xn2"] = es_xn2 = ExitStack()
        p_xn2 = es_xn2.enter_context(tc.tile_pool(name="p_xn2", bufs=1))
        S["xn2_sb"] = p_xn2.tile([P, NC, 1024], BF16, tag="xn2", name="xn2")
        wo_ln2(0)

        # ---------------- Phase G: slot1 attention woven with W1(slot0) ---
        _mark(nc, "attn1")
        _weave(attn_units(1), w1_units(0), fill_every=4, fill_n=1)

        # ---------------- Phase H: slot1 wo + LN2; W1(slot1); W2 ----------
        _mark(nc, "wo1")
        wo_ln2(1)
        es_masks.close()
        es_kqv.close()
        _mark(nc, "mlp")
        for _ in w1_units(1):
            pass
        S["p_xn2"].close()
        S["p_wo"].close()
        S["p_out"] = ctx.enter_context(tc.tile_pool(name="p_out", bufs=3))
        for _ in w2_units(0):
            pass
        for _ in w2_units(1):
            pass

    nc.compile()
    return nc


def _host_inputs(X, w_q, w_k, w_v, w_o, W1, b1, W2, b2, g1, be1, g2, be2):
    """Build the 8 per-core input dicts."""
    f32 = np.float32
    import ml_dtypes
    bf16 = ml_dtypes.bfloat16
    wqt = np.ascontiguousarray(np.asarray(w_q, f32).reshape(D, D).T.astype(bf16))
    wkt = np.ascontiguousarray(np.asarray(w_k, f32).reshape(D, D).T.astype(bf16))
    wvt = np.ascontiguousarray(np.asarray(w_v, f32).reshape(D, D).T.astype(bf16))
    wo = np.ascontiguousarray(np.asarray(w_o, f32).astype(bf16))
    onesbc = np.ones((P, P), f32)
    # 4 canonical self-diagonal masks: mask[k][s, t] = (128k + s <= t)
    masks = np.zeros((4, P, TB), bf16)
    ar_s = np.arange(P)[:, None]
    ar_t = np.arange(TB)[None, :]
    for k in range(4):
        masks[k] = (128 * k + ar_s <= ar_t).astype(bf16)
    w1t = np.ascontiguousarray(np.asarray(W1, f32).T.astype(bf16))
    w2t = np.ascontiguousarray(np.asarray(W2, f32).T.astype(bf16))

    # per-role exp scale/bias: 24 = 8 (slot0) + 16 (slot1) chunk positions
    sc = {}
    bi = {}
    for role in range(2):
        order = ROLE_ORDER[role]
        s = np.full((24,), 0.125, f32)
        b = np.zeros((24,), f32)
        for sl_i in range(2):
            own_blk = order[sl_i]
            for ci, ch in enumerate(SLOT_CHUNKS[sl_i]):
                idx = (0 if sl_i == 0 else 8) + ci
                pos = ch // 4           # permuted 512-block of this s-chunk
                blk = order[pos]
                if pos == sl_i or blk < own_blk:
                    pass                # diagonal (tri-masked) or past: live
                else:
                    s[idx] = 0.0        # future: dead
                    b[idx] = DEAD
        sc[role] = np.broadcast_to(s, (P, 24)).copy()
        bi[role] = np.broadcast_to(b, (P, 24)).copy()

    shared = dict(wqt=wqt, wkt=wkt, wvt=wvt, wo=wo, w1t=w1t, w2t=w2t,
                  onesbc=onesbc, masks=masks,
                  g1v=np.asarray(g1, f32), be1v=np.asarray(be1, f32),
                  g2v=np.asarray(g2, f32), be2v=np.asarray(be2, f32),
                  b1v=np.asarray(b1, f32), b2v=np.asarray(b2, f32))

    in_maps = []
    for core in range(8):
        role, b_idx = core // 4, core % 4
        order = ROLE_ORDER[role]
        xb = np.asarray(X[b_idx], f32)          # [T, D]
        xperm = np.concatenate([xb[o * TB:(o + 1) * TB] for o in order], axis=0)
        xt = np.ascontiguousarray(xperm.T)      # [D, T]
        m = dict(shared)
        m["xt"] = xt
        m["scalein"] = sc[role]
        m["biasin"] = bi[role]
        in_maps.append(m)
    return in_maps


def _assemble(results, dtype):
    out = np.empty((B, T, D), dtype)
    for core in range(8):
        role, b_idx = core // 4, core % 4
        order = ROLE_ORDER[role]
        ot = results[core]["outt"]              # [D, 1024]
        for sl_i in range(2):
            blk = order[sl_i]
            out[b_idx, blk * TB:(blk + 1) * TB] = \
                ot[:, sl_i * TB:(sl_i + 1) * TB].T
    return out


def kernel(X, w_q, w_k, w_v, w_o, W1, b1, W2, b2, g1, be1, g2, be2,
           _want_results=False, _trace=False):
    if "nc" not in _cached:
        _cached["nc"] = _build_nc()
    nc = _cached["nc"]
    in_maps = _host_inputs(X, w_q, w_k, w_v, w_o, W1, b1, W2, b2,
                           g1, be1, g2, be2)
    res = run_bass_kernel_spmd(nc, in_maps, core_ids=list(range(8)),
                               trace=_trace)
    out = _assemble(res.results, np.asarray(X).dtype)
    if _want_results:
        return out, res
    return out


# revision 19
# speedup vs baseline: 1.1284x; 1.1284x over previous
"""Trainium2 Bass kernel for a dense transformer block (pre-LN attn + MLP).

B=4, T=2048, D=768, H=12 (DH=64), DFF=3072, fp32.

Sharding: 8 cores = 4 batches x 2 roles. Each core processes one batch and
owns 1024 query tokens (two 512-blocks, paired {0,3}/{1,2} for causal load
balance). K/V are computed for the full 2048 tokens on both cores of a batch
(cheap), so there are NO collectives.

SPMD uniformity: all 8 cores run ONE identical NEFF. Causal structure is
carried in DATA, not code:
  - host permutes each batch's token axis to [own0, own1, otherA, otherB]
  - q-slot0 attends s-chunks {0..3, 8..11}; q-slot1 attends s-chunks {0..15}
  - per-(slot,chunk) exp scale/bias inputs select live / dead (zero) chunks
  - 4 canonical triangular masks handle the self-diagonal 512-blocks

Everything on-chip runs in a transposed layout (features on partitions,
tokens on the free axis). Matmul cost on the PE depends only on the output
free size, so per-token LayerNorm statistics are computed REPLICATED across
all 128 partitions (ones [128,128] lhsT) and softmax denominators come
replicated across 64 partitions from a shared ones-block appended to V —
no partition broadcasts are needed anywhere.

Engine assignment: PE does GEMMs + LN column-sum stats; Act does exp and the
LN apply (scale/bias copy); DVE does LN tensor ops, masks, softmax divides
and residuals; Pool (gpsimd) does all PSUM->SBUF copies, squares and relu.
Emission is software-pipelined: slot0 attention is woven with the remaining
QKV projections, slot1 attention with the first half of the MLP, so the PE
never idles behind the Act exp chain.
"""

import sys

sys.path.insert(0, "/opt/trn_rl_repo")

from contextlib import ExitStack

import numpy as np

import concourse.bass as bass
import concourse.mybir as mybir
import concourse.tile as tile
from concourse import bacc
from concourse.bass_utils import run_bass_kernel_spmd

F32 = mybir.dt.float32
E4 = mybir.dt.float8e4
F32R = mybir.dt.float32r
AF = mybir.ActivationFunctionType
BF16 = mybir.dt.bfloat16
ALU = mybir.AluOpType

H, D, DFF = 12, 768, 3072
DH = 64
B, T = 4, 2048
EPS = 1e-5
P = 128
NC = D // P          # 6 feature chunks
NF = DFF // P        # 24 ff tiles
TB = 512             # token block
NTB = T // TB        # 4 blocks
VW = H * 65          # 780: per head 64 values + 1 ones col
SLOT_CHUNKS = [[0, 1, 2, 3, 8, 9, 10, 11], list(range(16))]
# role -> permuted block order [own0, own1, restA, restB] (original block ids)
ROLE_ORDER = [[0, 3, 1, 2], [1, 2, 0, 3]]
DEAD = -30000.0      # exp(DEAD) == 0 in fp32
WS = 32.0            # fp8 weight scale; products carry WS, descaled at epilogue
DS = 1.0 / WS
MM = mybir.MatmulPerfMode.DoubleRow
TBORD = [0, 2, 1, 3]  # emission order: slot0 needs permuted blocks 0 and 2

_cached = {}
PHASE_MARKS = []


def _mark(nc, name):
    PHASE_MARKS.append((name, nc.next_id()))


def _weave(primary, filler, fill_every=1, fill_n=1):
    """Drain primary generator; after every `fill_every` primary units, pull
    `fill_n` units from filler. Any leftover filler drains at the end."""
    i = 0
    f = iter(filler)
    for _ in primary:
        i += 1
        if i % fill_every == 0:
            for _ in range(fill_n):
                if next(f, StopIteration) is StopIteration:
                    break
    for _ in f:
        pass


def _build_nc():
    nc = bacc.Bacc("TRN2", target_bir_lowering=False, debug=False,
                   enable_asserts=False, num_devices=8)

    def din(name, shape, dt=F32R):
        return nc.dram_tensor(name, shape, dt, kind="ExternalInput").ap()

    xt_d = din("xt", [D, T])                 # X[b].T, token-permuted
    wq8_d = din("wq8", [D, D], E4)           # w_q as [c, m], x32
    wq8r_d = din("wq8r", [D, D], E4)         # residual, x32
    wk8_d = din("wk8", [D, D], E4)
    wk8r_d = din("wk8r", [D, D], E4)
    wv8_d = din("wv8", [D, D], E4)
    wv8r_d = din("wv8r", [D, D], E4)
    wo_d = din("wo", [D, D], BF16)           # natural [m, c]
    w18_d = din("w18", [D, DFF], E4)         # W1.T  [c, f], x32
    w18r_d = din("w18r", [D, DFF], E4)
    w28_d = din("w28", [DFF, D], E4)         # W2.T  [f, c], x32
    w28r_d = din("w28r", [DFF, D], E4)
    onesbc_d = din("onesbc", [P, P])         # all-ones lhsT: replicated sums
    masks_d = din("masks", [4, P, TB], BF16)  # tri masks
    scalein_d = din("scalein", [P, 24], F32) # exp scale per (slot,chunk)
    biasin_d = din("biasin", [P, 24], F32)   # exp bias per (slot,chunk)
    g1_d = din("g1v", [D], F32)
    be1_d = din("be1v", [D], F32)
    g2_d = din("g2v", [D], F32)
    be2_d = din("be2v", [D], F32)
    b1_d = din("b1v", [DFF], F32)
    b2_d = din("b2v", [D], F32)

    outt_d = nc.dram_tensor("outt", [D, 1024], F32, kind="ExternalOutput").ap()

    xt_r = xt_d.rearrange("(j p) t -> p j t", p=P)

    with tile.TileContext(nc) as tc, ExitStack() as ctx, \
         nc.allow_low_precision(reason="fp32r/bf16 intermediates are intended"):
        consts = ctx.enter_context(tc.tile_pool(name="consts", bufs=1))
        ps = ctx.enter_context(tc.tile_pool(name="ps", bufs=1, space="PSUM"))
        work = ctx.enter_context(tc.tile_pool(name="work", bufs=2))
        p_xp = ctx.enter_context(tc.tile_pool(name="p_xp", bufs=1))
        xp_sb = p_xp.tile([P, NC, 1024], F32R, tag="xp", name="xp")
        p_yt = ctx.enter_context(tc.tile_pool(name="p_yt", bufs=1))
        yt_all = p_yt.tile([P, NC, TB], BF16, tag="yt_all", name="yt_all")
        S = {"xp_sb": xp_sb}

        onesbc_sb = consts.tile([P, P], F32R, tag="onesbc")
        scale_sb = consts.tile([P, 24], F32, tag="scalein")
        bias_sb = consts.tile([P, 24], F32, tag="biasin")
        g1_sb = consts.tile([P, NC], F32, tag="g1")
        be1_sb = consts.tile([P, NC], F32, tag="be1")
        g2_sb = consts.tile([P, NC], F32, tag="g2")
        be2_sb = consts.tile([P, NC], F32, tag="be2")
        b1_sb = consts.tile([P, NF], F32, tag="b1")
        b2_sb = consts.tile([P, NC], F32, tag="b2")

        def _early_const_dmas():
            nc.sync.dma_start(onesbc_sb[:], onesbc_d)
            for sb, d in ((g1_sb, g1_d), (be1_sb, be1_d)):
                nc.sync.dma_start(sb[:], d.rearrange("(j p) -> p j", p=P))

        def _late_const_dmas():
            nc.sync.dma_start(scale_sb[:], scalein_d)
            nc.sync.dma_start(bias_sb[:], biasin_d)
            for sb, d in ((g2_sb, g2_d), (be2_sb, be2_d)):
                nc.sync.dma_start(sb[:], d.rearrange("(j p) -> p j", p=P))
            nc.sync.dma_start(b1_sb[:], b1_d.rearrange("(j p) -> p j", p=P))
            nc.sync.dma_start(b2_sb[:], b2_d.rearrange("(j p) -> p j", p=P))
            nc.sync.dma_start(masks_sb[:], masks_d.rearrange("o p t -> p o t"))

        # ---------------- LayerNorm helpers ----------------
        def ln_stats(src3):
            """src3: [128, NC, TB] slice. Returns (r, mur): [128, TB] tiles
            with 1/std and mu/std replicated across partitions."""
            s1 = ps.tile([P, TB], F32, tag="acc", bufs=2, name="s1")
            s2 = ps.tile([P, TB], F32, tag="acc", bufs=2, name="s2")
            for j in range(NC):
                nc.tensor.matmul(s1[:], onesbc_sb[:], src3[:, j, :],
                                 start=(j == 0), stop=(j == NC - 1))
            for j in range(NC):
                sq = work.tile([P, TB], F32R, tag="sq", bufs=2)
                if j % 2 == 0:
                    nc.scalar.activation(sq[:], src3[:, j, :], AF.Square)
                else:
                    nc.gpsimd.tensor_mul(sq[:], src3[:, j, :], src3[:, j, :])
                nc.tensor.matmul(s2[:], onesbc_sb[:], sq[:],
                                 start=(j == 0), stop=(j == NC - 1))
            mu = work.tile([P, TB], F32, tag="mu", bufs=1)
            t = work.tile([P, TB], F32, tag="tmp", bufs=1)
            r = work.tile([P, TB], F32R, tag="r", bufs=2)
            mur = work.tile([P, TB], F32R, tag="mur", bufs=2)
            nc.vector.tensor_scalar_mul(mu[:], s1[:], 1.0 / D)
            nc.vector.tensor_mul(t[:], mu[:], mu[:])
            nc.vector.scalar_tensor_tensor(t[:], s2[:], 1.0 / D, t[:],
                                           ALU.mult, ALU.subtract)
            nc.vector.tensor_scalar_add(t[:], t[:], EPS)
            nc.scalar.activation(t[:], t[:], AF.Sqrt)
            nc.vector.reciprocal(r[:], t[:])
            nc.vector.tensor_mul(mur[:], mu[:], r[:])
            return r, mur

        def ln_norm_chunk(src_j, dst_j, r, mur, g_sb, be_sb, j):
            """dst = (src*r - mur)*g[p] + be[p]."""
            t1 = work.tile([P, TB], F32R, tag="nrm1")
            nc.vector.tensor_mul(t1[:], src_j, r[:])
            nc.vector.tensor_sub(t1[:], t1[:], mur[:])
            nc.scalar.activation(dst_j, t1[:], AF.Identity,
                                 bias=be_sb[:, j:j + 1], scale=g_sb[:, j:j + 1])

        # ---------------- attention-phase SBUF tensors ----------------
        es_kqv = ExitStack()
        p_kqv = es_kqv.enter_context(tc.tile_pool(name="p_kqv", bufs=1,
                                                  side="right"))
        kt_sb = p_kqv.tile([P, NC, T], BF16, tag="kt")      # K^T [m, s]
        qt_sb = p_kqv.tile([P, NC, 1024], BF16, tag="qt")   # Q^T [m, t_own]
        v_sb = p_kqv.tile([P, 16, VW], BF16, tag="v")       # V_ext [s, 780]
        v_hv = v_sb.rearrange("p s (h e) -> p s h e", e=65)
        vd_view = v_hv[:, :, :, 0:64]
        nc.vector.memset(v_hv[:, :, :, 64:65], 1.0)

        es_masks = ExitStack()
        p_masks = es_masks.enter_context(tc.tile_pool(name="p_masks", bufs=1,
                                                      side="right"))
        p_e = es_masks.enter_context(tc.tile_pool(name="p_e", bufs=2,
                                                  side="right"))
        masks_sb = p_masks.tile([P, 4, TB], BF16, tag="masks")

        es_wqkv = ExitStack()
        p_wqkv = es_wqkv.enter_context(tc.tile_pool(name="p_wqkv", bufs=1,
                                                    side="right"))
        wq_sb = p_wqkv.tile([P, NC, D], E4, tag="wq")
        wqr_sb = p_wqkv.tile([P, NC, D], E4, tag="wqr")
        wk_sb = p_wqkv.tile([P, NC, D], E4, tag="wk")
        wkr_sb = p_wqkv.tile([P, NC, D], E4, tag="wkr")
        wv_sb = p_wqkv.tile([P, NC, D], E4, tag="wv")
        wvr_sb = p_wqkv.tile([P, NC, D], E4, tag="wvr")

        # ---------------- Phase A-D: LN1 + QKV over TBORD, pipelined -------
        _mark(nc, "lnq")
        es_xn1 = ExitStack()
        p_xn1 = es_xn1.enter_context(tc.tile_pool(name="p_xn1", bufs=2))
        xn1_tiles = {}

        def split8_chunk(xnb, x8_j, rx8_j):
            nc.scalar.activation(x8_j, xnb[:], AF.Copy)
            nc.gpsimd.tensor_sub(rx8_j, xnb[:], x8_j)

        def norm_units(tb):
            r, mur = stats_res[tb]
            x8_t = p_xn1.tile([P, NC, TB], E4, tag="x8", name=f"x8_{tb}")
            rx8_t = p_xn1.tile([P, NC, TB], E4, tag="rx8", name=f"rx8_{tb}")
            xn1_tiles[tb] = (x8_t, rx8_t)
            xt_t = xtr_tiles[tb]
            for j in range(NC):
                xnb = work.tile([P, TB], BF16, tag="xnb")
                ln_norm_chunk(xt_t[:, j, :], xnb[:], r, mur,
                              g1_sb, be1_sb, j)
                split8_chunk(xnb, x8_t[:, j, :], rx8_t[:, j, :])
                yield

        def mm3(acc, wsb, wrsb, x8, rx8, msl, n):
            """acc += (w + rw).T (x + rx), 3-term fp8 DoubleRow over j pairs."""
            first = True
            for wa, xa in ((wsb, x8), (wsb, rx8), (wrsb, x8)):
                for jp in range(NC // 2):
                    j2 = slice(2 * jp, 2 * jp + 2)
                    nc.tensor.matmul(acc[:, :n], wa[:, j2, msl], xa[:, j2, :],
                                     start=first,
                                     stop=(wa is wrsb and jp == NC // 2 - 1),
                                     perf_mode=MM)
                    first = False

        def mm3t(acc, x8, rx8, ssl, wsb, wrsb, fsl, n):
            """acc += (x + rx).T (w + rw): x stationary variant."""
            first = True
            for xa, wa in ((x8, wsb), (x8, wrsb), (rx8, wsb)):
                for jp in range(NC // 2):
                    j2 = slice(2 * jp, 2 * jp + 2)
                    nc.tensor.matmul(acc[:, :n], xa[:, j2, ssl], wa[:, j2, fsl],
                                     start=first,
                                     stop=(xa is rx8 and jp == NC // 2 - 1),
                                     perf_mode=MM)
                    first = False

        def qkv_units(tb, want_q):
            tsl = slice(tb * TB, (tb + 1) * TB)
            x8_t, rx8_t = xn1_tiles[tb]
            for mt in range(NC):
                msl = slice(mt * P, (mt + 1) * P)
                acc = ps.tile([P, TB], F32, tag="acc", bufs=2, name="ka")
                mm3(acc, wk_sb, wkr_sb, x8_t, rx8_t, msl, TB)
                nc.vector.tensor_scalar_mul(kt_sb[:, mt, tsl], acc[:], DS)
                yield
            for st in range(4):
                ssl = slice(st * P, (st + 1) * P)
                for half, fsl, w in ((0, slice(0, TB), TB),
                                     (1, slice(TB, D), D - TB)):
                    acc = ps.tile([P, TB], F32, tag="acc", bufs=2, name="va")
                    mm3t(acc, x8_t, rx8_t, ssl, wv_sb, wvr_sb, fsl, w)
                    src = acc[:, :w].rearrange("p (h e) -> p h e", e=64)
                    h0 = half * 8
                    dst = vd_view[:, tb * 4 + st, h0:h0 + w // 64, :]
                    nc.vector.tensor_scalar_mul(dst, src, DS)
                    yield
            if want_q:
                for mt in range(NC):
                    msl = slice(mt * P, (mt + 1) * P)
                    acc = ps.tile([P, TB], F32, tag="acc", bufs=2, name="qa")
                    mm3(acc, wq_sb, wqr_sb, x8_t, rx8_t, msl, TB)
                    nc.vector.tensor_scalar_mul(qt_sb[:, mt, tsl], acc[:], DS)
                    yield

        es_xtr = ExitStack()
        p_xtr = es_xtr.enter_context(tc.tile_pool(name="p_xtr", bufs=2))
        xtr_tiles = {}
        stats_res = {}

        def load_xt(tb, step=2):
            tsl = slice(tb * TB, (tb + 1) * TB)
            xt_t = p_xtr.tile([P, NC, TB], F32R, tag="xtr", name=f"xt_{tb}")
            xtr_tiles[tb] = xt_t
            for j0 in range(0, NC, step):
                nc.sync.dma_start(xt_t[:, j0:j0 + step, :],
                                  xt_r[:, j0:j0 + step, tsl])

        # A: tb0 stats
        _early_const_dmas()
        load_xt(0, step=1)
        for sb, d in ((wk_sb, wk8_d), (wkr_sb, wk8r_d), (wv_sb, wv8_d),
                      (wvr_sb, wv8r_d), (wq_sb, wq8_d), (wqr_sb, wq8r_d)):
            nc.sync.dma_start(sb[:], d.rearrange("(j p) m -> p j m", p=P))
        stats_res[0] = ln_stats(xtr_tiles[0][:])
        # B: tb2 stats; norm0 + qkv0
        load_xt(2)
        _late_const_dmas()
        stats_res[2] = ln_stats(xtr_tiles[2][:])
        for _ in norm_units(0):
            pass
        for _ in qkv_units(0, want_q=True):
            pass
        # C: tb1 stats; norm2 + qkv2
        load_xt(1)
        stats_res[1] = ln_stats(xtr_tiles[1][:])
        for _ in norm_units(2):
            pass
        for _ in qkv_units(2, want_q=False):
            pass
        # D: tb3 stats; norm1
        load_xt(3)
        stats_res[3] = ln_stats(xtr_tiles[3][:])
        for _ in norm_units(1):
            pass

        # ---------------- attention machinery ----------------

        def attn_units(sl_i):
            qsl = slice(sl_i * TB, (sl_i + 1) * TB)
            chunks = SLOT_CHUNKS[sl_i]
            last_ci = len(chunks) - 1
            for mt in range(NC):
                yt2 = [ps.tile([65, TB], F32, tag="yt", bufs=2,
                               name=f"yt{ph}") for ph in range(2)]

                def pv(ci, e_sb):
                    ch = chunks[ci]
                    for ph in range(2):
                        h = 2 * mt + ph
                        nc.tensor.matmul(
                            yt2[ph][:], v_sb[:, ch, h * 65:(h + 1) * 65],
                            e_sb[:, ph * TB:(ph + 1) * TB],
                            start=(ci == 0), stop=(ci == last_ci))

                pend = None
                for ci, ch in enumerate(chunks):
                    sb_idx = (0 if sl_i == 0 else 8) + ci
                    st2 = ps.tile([P, 2 * TB], F32, tag="st", bufs=2)
                    for ph in range(2):
                        o = ph * 64
                        nc.tensor.matmul(
                            st2[:, ph * TB:(ph + 1) * TB],
                            kt_sb[o:o + 64, mt, ch * P:(ch + 1) * P],
                            qt_sb[o:o + 64, mt, qsl], start=True, stop=True)
                    e_sb = p_e.tile([P, 2 * TB], BF16, tag="e")
                    nc.scalar.activation(
                        e_sb[:], st2[:], AF.Exp,
                        bias=bias_sb[:, sb_idx:sb_idx + 1],
                        scale=scale_sb[:, sb_idx:sb_idx + 1])
                    di = ch - 4 * sl_i
                    if 0 <= di < 4:
                        for ph in range(2):
                            nc.vector.tensor_mul(
                                e_sb[:, ph * TB:(ph + 1) * TB],
                                e_sb[:, ph * TB:(ph + 1) * TB],
                                masks_sb[:, di, :])
                    if pend is not None:
                        pv(*pend)
                    pend = (ci, e_sb)
                    yield
                pv(*pend)
                for ph in range(2):
                    o = ph * 64
                    rc = work.tile([1, TB], F32R, tag="rc")
                    nc.vector.reciprocal(rc[:], yt2[ph][64:65, :])
                    bc = ps.tile([64, TB], F32, tag="acc", bufs=2, name="bc")
                    nc.tensor.matmul(bc[:], onesbc_sb[0:1, 0:64], rc[:],
                                     start=True, stop=True)
                    nc.vector.tensor_copy(yt_all[o:o + 64, mt, :],
                                          yt2[ph][0:64, :])
                    nc.vector.tensor_mul(yt_all[o:o + 64, mt, :],
                                         yt_all[o:o + 64, mt, :], bc[:])
                yield

        def wo_ln2(sl_i):
            qsl = slice(sl_i * TB, (sl_i + 1) * TB)
            x28_sb, rx28_sb = S["xn2_sb"]
            wo_sb = S["wo_sb"]
            for ct in range(NC):
                ao = ps.tile([P, TB], F32, tag="acc", bufs=2, name="ao")
                for mc in range(NC):
                    nc.tensor.matmul(ao[:], wo_sb[:, mc, ct * P:(ct + 1) * P],
                                     yt_all[:, mc, :],
                                     start=(mc == 0), stop=(mc == NC - 1))
                nc.vector.tensor_add(xp_sb[:, ct, qsl],
                                     xp_sb[:, ct, qsl], ao[:])
            r2, mur2 = ln_stats(xp_sb[:, :, qsl])
            for j in range(NC):
                xnb = work.tile([P, TB], BF16, tag="xnb")
                ln_norm_chunk(xp_sb[:, j, qsl], xnb[:], r2, mur2,
                              g2_sb, be2_sb, j)
                split8_chunk(xnb, x28_sb[:, j, qsl], rx28_sb[:, j, qsl])

        # ---------------- MLP machinery ----------------
        w18_r = w18_d.rearrange("(j p) f -> p j f", p=P)
        w18r_r = w18r_d.rearrange("(j p) f -> p j f", p=P)
        w28_r = w28_d.rearrange("(f p) c -> p f c", p=P)
        w28r_r = w28r_d.rearrange("(f p) c -> p f c", p=P)
        outt_r = outt_d.rearrange("(j p) t -> p j t", p=P)

        def w1_units(sl_i):
            tsl = slice(sl_i * TB, (sl_i + 1) * TB)
            (h8_sb, rh8_sb), p_wmlp = S["h1_sb"], S["p_wmlp"]
            x28_sb, rx28_sb = S["xn2_sb"]

            def w1_dma(ft2):
                fs = slice(2 * ft2 * P, (2 * ft2 + 2) * P)
                w1_t = p_wmlp.tile([P, NC, 2 * P], E4, tag="w1",
                                   name=f"w1_{sl_i}_{ft2}")
                w1r_t = p_wmlp.tile([P, NC, 2 * P], E4, tag="w1r",
                                    name=f"w1r_{sl_i}_{ft2}")
                nc.sync.dma_start(w1_t[:], w18_r[:, :, fs])
                nc.sync.dma_start(w1r_t[:], w18r_r[:, :, fs])
                return w1_t, w1r_t
            nxt = w1_dma(0)
            for ft2 in range(NF // 2):
                w1_t, w1r_t = nxt
                if ft2 + 1 < NF // 2:
                    nxt = w1_dma(ft2 + 1)
                for sub in range(2):
                    ft = 2 * ft2 + sub
                    msl = slice(sub * P, (sub + 1) * P)
                    hp = ps.tile([P, TB], F32, tag="acc", bufs=2, name="hp")
                    mm3(hp, w1_t, w1r_t, x28_sb[:, :, tsl],
                        rx28_sb[:, :, tsl], msl, TB)
                    hb = work.tile([P, TB], F32R, tag="nrm1", name="hb")
                    nc.vector.tensor_scalar(hb[:], hp[:], DS,
                                            b1_sb[:, ft:ft + 1],
                                            ALU.mult, ALU.add)
                    hbf = work.tile([P, TB], BF16, tag="xnb", name="hbf")
                    nc.vector.tensor_scalar_max(hbf[:], hb[:], 0.0)
                    nc.vector.tensor_copy(h8_sb[:, ft, tsl], hbf[:])
                    nc.gpsimd.tensor_sub(rh8_sb[:, ft, tsl], hbf[:],
                                         h8_sb[:, ft, tsl])
                    yield

        def w2_units(g):
            (h8_sb, rh8_sb), p_wmlp = S["h1_sb"], S["p_wmlp"]
            p_out = S["p_out"]
            o2s = {}
            for ci, (t, bu) in enumerate((("acc", 2), ("yt", 2))):
                for tb in range(2):
                    o2s[(ci, tb)] = ps.tile([P, TB], F32, tag=t, bufs=bu,
                                            name=f"o2_{g}_{ci}_{tb}")
            stp = ps.tile([P, 2 * TB], F32, tag="st", bufs=2,
                          name=f"o2st_{g}")
            o2s[(2, 0)] = stp[:, 0:TB]
            o2s[(2, 1)] = stp[:, TB:2 * TB]

            def w2_dma(ft2):
                fs = slice(2 * ft2, 2 * ft2 + 2)
                w2_t = p_wmlp.tile([P, 2, D], E4, tag="w2",
                                   name=f"w2_{g}_{ft2}")
                w2r_t = p_wmlp.tile([P, 2, D], E4, tag="w2r",
                                    name=f"w2r_{g}_{ft2}")
                nc.sync.dma_start(w2_t[:], w28_r[:, fs, :])
                nc.sync.dma_start(w2r_t[:], w28r_r[:, fs, :])
                return w2_t, w2r_t
            nxt = w2_dma(0)
            for ft2 in range(NF // 2):
                w2_t, w2r_t = nxt
                if ft2 + 1 < NF // 2:
                    nxt = w2_dma(ft2 + 1)
                for ci in range(3):
                    ct = g * 3 + ci
                    csl = slice(ct * P, (ct + 1) * P)
                    for tb in range(2):
                        hsl = slice(tb * TB, (tb + 1) * TB)
                        for wa, ha in ((w2_t, h8_sb), (w2_t, rh8_sb),
                                       (w2r_t, h8_sb)):
                            nc.tensor.matmul(
                                o2s[(ci, tb)][:], wa[:, :, csl],
                                ha[:, 2 * ft2:2 * ft2 + 2, hsl],
                                start=(ft2 == 0 and wa is w2_t
                                       and ha is h8_sb),
                                stop=(ft2 == NF // 2 - 1 and wa is w2r_t),
                                perf_mode=MM)
                yield
            for tb in range(2):
                tsl = slice(tb * TB, (tb + 1) * TB)
                for ci in range(3):
                    ct = g * 3 + ci
                    xpb = work.tile([P, TB], F32R, tag="nrm1", name="xpb")
                    nc.vector.tensor_scalar(xpb[:], xp_sb[:, ct, tsl],
                                            b2_sb[:, ct:ct + 1], None,
                                            ALU.add)
                    ot = p_out.tile([P, TB], F32, tag="ot")
                    nc.vector.scalar_tensor_tensor(
                        ot[:], o2s[(ci, tb)][:], DS, xpb[:],
                        ALU.mult, ALU.add)
                    nc.sync.dma_start(outt_r[:, ct, tsl], ot[:])
                    yield

        # ---------------- Phase E: slot0 attention woven with rest --------
        _mark(nc, "attn0")

        def e_fill():
            for u in norm_units(3):
                yield u
            es_xtr.close()
            for u in qkv_units(1, want_q=True):
                yield u
            for u in qkv_units(3, want_q=False):
                yield u
            es_xn1.close()
            es_wqkv.close()
            p_h1 = ctx.enter_context(tc.tile_pool(name="p_h1", bufs=1))
            S["h1_sb"] = (
                p_h1.tile([P, NF, 1024], E4, tag="h8", name="h8"),
                p_h1.tile([P, NF, 1024], E4, tag="rh8", name="rh8"))
            S["p_wmlp"] = ctx.enter_context(
                tc.tile_pool(name="p_wmlp", bufs=2))
            S["p_wo"] = es_wo = ExitStack()
            p_wo = es_wo.enter_context(tc.tile_pool(name="p_wo", bufs=1))
            S["wo_sb"] = wo_sb = p_wo.tile([P, NC, D], BF16, tag="wo", name="wo")
            nc.sync.dma_start(wo_sb[:],
                              wo_d.rearrange("(j p) m -> p j m", p=P))
            nc.sync.dma_start(xp_sb[:], xt_r[:, :, 0:1024])
            yield

        _weave(attn_units(0), e_fill(), fill_every=1, fill_n=1)

        # ---------------- Phase F: slot0 wo + LN2 ----------------
        _mark(nc, "wo0")
        S["p_xn2"] = es_xn2 = ExitStack()
        p_xn2 = es_xn2.enter_context(tc.tile_pool(name="p_xn2", bufs=1))
        S["xn2_sb"] = (
            p_xn2.tile([P, NC, 1024], E4, tag="x28", name="x28"),
            p_xn2.tile([P, NC, 1024], E4, tag="rx28", name="rx28"))
        wo_ln2(0)

        # ---------------- Phase G: slot1 attention woven with W1(slot0) ---
        _mark(nc, "attn1")
        _weave(attn_units(1), w1_units(0), fill_every=4, fill_n=1)

        # ---------------- Phase H: slot1 wo + LN2; W1(slot1); W2 ----------
        _mark(nc, "wo1")
        wo_ln2(1)
        es_masks.close()
        es_kqv.close()
        _mark(nc, "mlp")
        for _ in w1_units(1):
            pass
        S["p_xn2"].close()
        S["p_wo"].close()
        S["p_out"] = ctx.enter_context(tc.tile_pool(name="p_out", bufs=3))
        for _ in w2_units(0):
            pass
        for _ in w2_units(1):
            pass

    nc.compile()
    return nc


def _host_inputs(X, w_q, w_k, w_v, w_o, W1, b1, W2, b2, g1, be1, g2, be2):
    """Build the 8 per-core input dicts."""
    f32 = np.float32
    import ml_dtypes
    bf16 = ml_dtypes.bfloat16
    e4 = ml_dtypes.float8_e4m3

    def split8(w):
        ws = np.asarray(w, f32) * WS
        a = ws.astype(e4)
        r = (ws - a.astype(f32)).astype(e4)
        return np.ascontiguousarray(a), np.ascontiguousarray(r)

    wq8, wq8r = split8(np.asarray(w_q, f32).reshape(D, D).T)
    wk8, wk8r = split8(np.asarray(w_k, f32).reshape(D, D).T)
    wv8, wv8r = split8(np.asarray(w_v, f32).reshape(D, D).T)
    wo = np.ascontiguousarray(np.asarray(w_o, f32).astype(bf16))
    onesbc = np.ones((P, P), f32)
    # 4 canonical self-diagonal masks: mask[k][s, t] = (128k + s <= t)
    masks = np.zeros((4, P, TB), bf16)
    ar_s = np.arange(P)[:, None]
    ar_t = np.arange(TB)[None, :]
    for k in range(4):
        masks[k] = (128 * k + ar_s <= ar_t).astype(bf16)
    w18, w18r = split8(np.asarray(W1, f32).T)
    w28, w28r = split8(np.asarray(W2, f32).T)

    # per-role exp scale/bias: 24 = 8 (slot0) + 16 (slot1) chunk positions
    sc = {}
    bi = {}
    for role in range(2):
        order = ROLE_ORDER[role]
        s = np.full((24,), 0.125, f32)
        b = np.zeros((24,), f32)
        for sl_i in range(2):
            own_blk = order[sl_i]
            for ci, ch in enumerate(SLOT_CHUNKS[sl_i]):
                idx = (0 if sl_i == 0 else 8) + ci
                pos = ch // 4           # permuted 512-block of this s-chunk
                blk = order[pos]
                if pos == sl_i or blk < own_blk:
                    pass                # diagonal (tri-masked) or past: live
                else:
                    s[idx] = 0.0        # future: dead
                    b[idx] = DEAD
        sc[role] = np.broadcast_to(s, (P, 24)).copy()
        bi[role] = np.broadcast_to(b, (P, 24)).copy()

    shared = dict(wq8=wq8, wq8r=wq8r, wk8=wk8, wk8r=wk8r, wv8=wv8,
                  wv8r=wv8r, wo=wo, w18=w18, w18r=w18r, w28=w28, w28r=w28r,
                  onesbc=onesbc, masks=masks,
                  g1v=np.asarray(g1, f32), be1v=np.asarray(be1, f32),
                  g2v=np.asarray(g2, f32), be2v=np.asarray(be2, f32),
                  b1v=np.asarray(b1, f32), b2v=np.asarray(b2, f32))

    in_maps = []
    for core in range(8):
        role, b_idx = core // 4, core % 4
        order = ROLE_ORDER[role]
        xb = np.asarray(X[b_idx], f32)          # [T, D]
        xperm = np.concatenate([xb[o * TB:(o + 1) * TB] for o in order], axis=0)
        xt = np.ascontiguousarray(xperm.T)      # [D, T]
        m = dict(shared)
        m["xt"] = xt
        m["scalein"] = sc[role]
        m["biasin"] = bi[role]
        in_maps.append(m)
    return in_maps


def _assemble(results, dtype):
    out = np.empty((B, T, D), dtype)
    for core in range(8):
        role, b_idx = core // 4, core % 4
        order = ROLE_ORDER[role]
        ot = results[core]["outt"]              # [D, 1024]
        for sl_i in range(2):
            blk = order[sl_i]
            out[b_idx, blk * TB:(blk + 1) * TB] = \
                ot[:, sl_i * TB:(sl_i + 1) * TB].T
    return out


def kernel(X, w_q, w_k, w_v, w_o, W1, b1, W2, b2, g1, be1, g2, be2,
           _want_results=False, _trace=False):
    if "nc" not in _cached:
        _cached["nc"] = _build_nc()
    nc = _cached["nc"]
    in_maps = _host_inputs(X, w_q, w_k, w_v, w_o, W1, b1, W2, b2,
                           g1, be1, g2, be2)
    res = run_bass_kernel_spmd(nc, in_maps, core_ids=list(range(8)),
                               trace=_trace)
    out = _assemble(res.results, np.asarray(X).dtype)
    if _want_results:
        return out, res
    return out


# revision 23
# speedup vs baseline: 1.1529x; 1.0216x over previous
"""Trainium2 Bass kernel for a dense transformer block (pre-LN attn + MLP).

B=4, T=2048, D=768, H=12 (DH=64), DFF=3072, fp32.

Sharding: 8 cores = 4 batches x 2 roles. Each core processes one batch and
owns 1024 query tokens (two 512-blocks, paired {0,3}/{1,2} for causal load
balance). K/V are computed for the full 2048 tokens on both cores of a batch
(cheap), so there are NO collectives.

SPMD uniformity: all 8 cores run ONE identical NEFF. Causal structure is
carried in DATA, not code:
  - host permutes each batch's token axis to [own0, own1, otherA, otherB]
  - q-slot0 attends s-chunks {0..3, 8..11}; q-slot1 attends s-chunks {0..15}
  - per-(slot,chunk) exp scale/bias inputs select live / dead (zero) chunks
  - 4 canonical triangular masks handle the self-diagonal 512-blocks

Everything on-chip runs in a transposed layout (features on partitions,
tokens on the free axis). Matmul cost on the PE depends only on the output
free size, so per-token LayerNorm statistics are computed REPLICATED across
all 128 partitions (ones [128,128] lhsT) and softmax denominators come
replicated across 64 partitions from a shared ones-block appended to V —
no partition broadcasts are needed anywhere.

Engine assignment: PE does GEMMs + LN column-sum stats; Act does exp and the
LN apply (scale/bias copy); DVE does LN tensor ops, masks, softmax divides
and residuals; Pool (gpsimd) does all PSUM->SBUF copies, squares and relu.
Emission is software-pipelined: slot0 attention is woven with the remaining
QKV projections, slot1 attention with the first half of the MLP, so the PE
never idles behind the Act exp chain.
"""

import sys

sys.path.insert(0, "/opt/trn_rl_repo")

from contextlib import ExitStack

import numpy as np

import concourse.bass as bass
import concourse.mybir as mybir
import concourse.tile as tile
from concourse import bacc
from concourse.bass_utils import run_bass_kernel_spmd

F32 = mybir.dt.float32
E4 = mybir.dt.float8e4
F32R = mybir.dt.float32r
AF = mybir.ActivationFunctionType
BF16 = mybir.dt.bfloat16
ALU = mybir.AluOpType

H, D, DFF = 12, 768, 3072
DH = 64
B, T = 4, 2048
EPS = 1e-5
P = 128
NC = D // P          # 6 feature chunks
NF = DFF // P        # 24 ff tiles
TB = 512             # token block
NTB = T // TB        # 4 blocks
VW = H * 65          # 780: per head 64 values + 1 ones col
SLOT_CHUNKS = [[0, 1, 2, 3, 8, 9, 10, 11], list(range(16))]
# role -> permuted block order [own0, own1, restA, restB] (original block ids)
ROLE_ORDER = [[0, 3, 1, 2], [1, 2, 0, 3]]
DEAD = -30000.0      # exp(DEAD) == 0 in fp32
WS = 32.0            # fp8 weight scale; products carry WS, descaled at epilogue
DS = 1.0 / WS
MM = mybir.MatmulPerfMode.DoubleRow
TBORD = [0, 2, 1, 3]  # emission order: slot0 needs permuted blocks 0 and 2

_cached = {}
PHASE_MARKS = []


def _mark(nc, name):
    PHASE_MARKS.append((name, nc.next_id()))


def _weave(primary, filler, fill_per=1.0):
    """Drain primary generator; after each primary unit pull `fill_per`
    (fractional) units from filler. Leftover filler drains at the end."""
    acc = 0.0
    f = iter(filler)
    for _ in primary:
        acc += fill_per
        while acc >= 1.0:
            acc -= 1.0
            if next(f, StopIteration) is StopIteration:
                acc = 0.0
                break
    for _ in f:
        pass


def _build_nc():
    nc = bacc.Bacc("TRN2", target_bir_lowering=False, debug=False,
                   enable_asserts=False, num_devices=8)

    def din(name, shape, dt=F32R):
        return nc.dram_tensor(name, shape, dt, kind="ExternalInput").ap()

    xt_d = din("xt", [D, T])                 # X[b].T, token-permuted
    wq8_d = din("wq8", [D, D], E4)           # w_q as [c, m], x32
    wq8r_d = din("wq8r", [D, D], E4)         # residual, x32
    wk8_d = din("wk8", [D, D], E4)
    wk8r_d = din("wk8r", [D, D], E4)
    wv8_d = din("wv8", [D, D], E4)
    wv8r_d = din("wv8r", [D, D], E4)
    wo_d = din("wo", [D, D], BF16)           # natural [m, c]
    w18_d = din("w18", [D, DFF], E4)         # W1.T  [c, f], x32
    w18r_d = din("w18r", [D, DFF], E4)
    w28_d = din("w28", [DFF, D], E4)         # W2.T  [f, c], x32
    w28r_d = din("w28r", [DFF, D], E4)
    onesbc_d = din("onesbc", [P, P])         # all-ones lhsT: replicated sums
    masks_d = din("masks", [4, P, TB], BF16)  # tri masks
    scalein_d = din("scalein", [P, 24], F32) # exp scale per (slot,chunk)
    biasin_d = din("biasin", [P, 24], F32)   # exp bias per (slot,chunk)
    g1_d = din("g1v", [D], F32)
    be1_d = din("be1v", [D], F32)
    g2_d = din("g2v", [D], F32)
    be2_d = din("be2v", [D], F32)
    b1_d = din("b1v", [DFF], F32)
    b2_d = din("b2v", [D], F32)

    outt_d = nc.dram_tensor("outt", [D, 1024], F32, kind="ExternalOutput").ap()

    xt_r = xt_d.rearrange("(j p) t -> p j t", p=P)

    with tile.TileContext(nc) as tc, ExitStack() as ctx, \
         nc.allow_low_precision(reason="fp32r/bf16 intermediates are intended"):
        consts = ctx.enter_context(tc.tile_pool(name="consts", bufs=1))
        ps = ctx.enter_context(tc.tile_pool(name="ps", bufs=1, space="PSUM"))
        work = ctx.enter_context(tc.tile_pool(name="work", bufs=2))
        p_xp = ctx.enter_context(tc.tile_pool(name="p_xp", bufs=1))
        xp_sb = p_xp.tile([P, NC, 1024], F32R, tag="xp", name="xp")
        p_yt = ctx.enter_context(tc.tile_pool(name="p_yt", bufs=1))
        yt_all = p_yt.tile([P, NC, TB], BF16, tag="yt_all", name="yt_all")
        S = {"xp_sb": xp_sb}

        onesbc_sb = consts.tile([P, P], F32R, tag="onesbc")
        scale_sb = consts.tile([P, 24], F32, tag="scalein")
        bias_sb = consts.tile([P, 24], F32, tag="biasin")
        g1_sb = consts.tile([P, NC], F32, tag="g1")
        be1_sb = consts.tile([P, NC], F32, tag="be1")
        g2_sb = consts.tile([P, NC], F32, tag="g2")
        be2_sb = consts.tile([P, NC], F32, tag="be2")
        b1_sb = consts.tile([P, NF], F32, tag="b1")
        b2_sb = consts.tile([P, NC], F32, tag="b2")

        def _early_const_dmas():
            nc.sync.dma_start(onesbc_sb[:], onesbc_d)
            for sb, d in ((g1_sb, g1_d), (be1_sb, be1_d)):
                nc.sync.dma_start(sb[:], d.rearrange("(j p) -> p j", p=P))

        def _late_const_dmas():
            nc.sync.dma_start(scale_sb[:], scalein_d)
            nc.sync.dma_start(bias_sb[:], biasin_d)
            for sb, d in ((g2_sb, g2_d), (be2_sb, be2_d)):
                nc.sync.dma_start(sb[:], d.rearrange("(j p) -> p j", p=P))
            nc.sync.dma_start(b1_sb[:], b1_d.rearrange("(j p) -> p j", p=P))
            nc.sync.dma_start(b2_sb[:], b2_d.rearrange("(j p) -> p j", p=P))
            nc.sync.dma_start(masks_sb[:], masks_d.rearrange("o p t -> p o t"))

        # ---------------- LayerNorm helpers ----------------
        def ln_stats(src3, sq_pool=False):
            """src3: [128, NC, TB] slice. Returns (r, mur): [128, TB] tiles
            with 1/std and mu/std replicated across partitions."""
            s1 = ps.tile([P, TB], F32, tag="acc", bufs=2, name="s1")
            s2 = ps.tile([P, TB], F32, tag="acc", bufs=2, name="s2")
            for j in range(NC):
                nc.tensor.matmul(s1[:], onesbc_sb[:], src3[:, j, :],
                                 start=(j == 0), stop=(j == NC - 1))
            for j in range(NC):
                sq = work.tile([P, TB], F32R, tag="sq", bufs=2)
                if not sq_pool and j % 2 == 0:
                    nc.scalar.activation(sq[:], src3[:, j, :], AF.Square)
                else:
                    nc.gpsimd.tensor_mul(sq[:], src3[:, j, :], src3[:, j, :])
                nc.tensor.matmul(s2[:], onesbc_sb[:], sq[:],
                                 start=(j == 0), stop=(j == NC - 1))
            mu = work.tile([P, TB], F32, tag="mu", bufs=1)
            t = work.tile([P, TB], F32, tag="tmp", bufs=1)
            r = work.tile([P, TB], F32R, tag="r", bufs=2)
            mur = work.tile([P, TB], F32R, tag="mur", bufs=2)
            nc.vector.tensor_scalar_mul(mu[:], s1[:], 1.0 / D)
            nc.vector.tensor_mul(t[:], mu[:], mu[:])
            nc.vector.scalar_tensor_tensor(t[:], s2[:], 1.0 / D, t[:],
                                           ALU.mult, ALU.subtract)
            nc.vector.tensor_scalar_add(t[:], t[:], EPS)
            nc.scalar.activation(t[:], t[:], AF.Sqrt)
            nc.vector.reciprocal(r[:], t[:])
            nc.vector.tensor_mul(mur[:], mu[:], r[:])
            return r, mur

        def ln_norm_chunk(src_j, dst_j, r, mur, g_sb, be_sb, j):
            """dst = (src*r - mur)*g[p] + be[p]."""
            t1 = work.tile([P, TB], F32R, tag="nrm1")
            nc.vector.tensor_mul(t1[:], src_j, r[:])
            nc.vector.tensor_sub(t1[:], t1[:], mur[:])
            nc.scalar.activation(dst_j, t1[:], AF.Identity,
                                 bias=be_sb[:, j:j + 1], scale=g_sb[:, j:j + 1])

        # ---------------- attention-phase SBUF tensors ----------------
        es_kqv = ExitStack()
        p_kqv = es_kqv.enter_context(tc.tile_pool(name="p_kqv", bufs=1,
                                                  side="right"))
        kt_sb = p_kqv.tile([P, NC, T], BF16, tag="kt")      # K^T [m, s]
        qt_sb = p_kqv.tile([P, NC, 1024], BF16, tag="qt")   # Q^T [m, t_own]
        v_sb = p_kqv.tile([P, 16, VW], BF16, tag="v")       # V_ext [s, 780]
        v_hv = v_sb.rearrange("p s (h e) -> p s h e", e=65)
        vd_view = v_hv[:, :, :, 0:64]
        nc.vector.memset(v_hv[:, :, :, 64:65], 1.0)

        es_masks = ExitStack()
        p_masks = es_masks.enter_context(tc.tile_pool(name="p_masks", bufs=1,
                                                      side="right"))
        p_e = es_masks.enter_context(tc.tile_pool(name="p_e", bufs=2,
                                                  side="right"))
        masks_sb = p_masks.tile([P, 4, TB], BF16, tag="masks")

        es_wqkv = ExitStack()
        p_wqkv = es_wqkv.enter_context(tc.tile_pool(name="p_wqkv", bufs=1,
                                                    side="right"))
        wq_sb = p_wqkv.tile([P, NC, D], E4, tag="wq")
        wqr_sb = p_wqkv.tile([P, NC, D], E4, tag="wqr")
        wk_sb = p_wqkv.tile([P, NC, D], E4, tag="wk")
        wkr_sb = p_wqkv.tile([P, NC, D], E4, tag="wkr")
        wv_sb = p_wqkv.tile([P, NC, D], E4, tag="wv")
        wvr_sb = p_wqkv.tile([P, NC, D], E4, tag="wvr")

        # ---------------- Phase A-D: LN1 + QKV over TBORD, pipelined -------
        _mark(nc, "lnq")
        es_xn1 = ExitStack()
        p_xn1 = es_xn1.enter_context(tc.tile_pool(name="p_xn1", bufs=2))
        xn1_tiles = {}

        def split8_chunk(xnb, x8_j, rx8_j, j=0):
            nc.scalar.activation(x8_j, xnb[:], AF.Copy)
            if j % 2 == 0:
                nc.vector.tensor_sub(rx8_j, xnb[:], x8_j)
            else:
                nc.gpsimd.tensor_sub(rx8_j, xnb[:], x8_j)

        def norm_units(tb):
            r, mur = stats_res[tb]
            x8_t = p_xn1.tile([P, NC, TB], E4, tag="x8", name=f"x8_{tb}")
            rx8_t = p_xn1.tile([P, NC, TB], E4, tag="rx8", name=f"rx8_{tb}")
            xn1_tiles[tb] = (x8_t, rx8_t)
            xt_t = xtr_tiles[tb]
            for j in range(NC):
                xnb = work.tile([P, TB], BF16, tag="xnb")
                ln_norm_chunk(xt_t[:, j, :], xnb[:], r, mur,
                              g1_sb, be1_sb, j)
                split8_chunk(xnb, x8_t[:, j, :], rx8_t[:, j, :], j)
                yield

        def mm3(acc, wsb, wrsb, x8, rx8, msl, n):
            """acc += (w + rw).T (x + rx), 3-term fp8 DoubleRow over j pairs."""
            first = True
            for wa, xa in ((wsb, x8), (wsb, rx8), (wrsb, x8)):
                for jp in range(NC // 2):
                    j2 = slice(2 * jp, 2 * jp + 2)
                    nc.tensor.matmul(acc[:, :n], wa[:, j2, msl], xa[:, j2, :],
                                     start=first,
                                     stop=(wa is wrsb and jp == NC // 2 - 1),
                                     perf_mode=MM)
                    first = False

        def mm3t(acc, x8, rx8, ssl, wsb, wrsb, fsl, n):
            """acc += (x + rx).T (w + rw): x stationary variant."""
            first = True
            for xa, wa in ((x8, wsb), (x8, wrsb), (rx8, wsb)):
                for jp in range(NC // 2):
                    j2 = slice(2 * jp, 2 * jp + 2)
                    nc.tensor.matmul(acc[:, :n], xa[:, j2, ssl], wa[:, j2, fsl],
                                     start=first,
                                     stop=(xa is rx8 and jp == NC // 2 - 1),
                                     perf_mode=MM)
                    first = False

        def qkv_units(tb, want_q):
            tsl = slice(tb * TB, (tb + 1) * TB)
            x8_t, rx8_t = xn1_tiles[tb]
            for mt in range(NC):
                msl = slice(mt * P, (mt + 1) * P)
                acc = ps.tile([P, TB], F32, tag="acc", bufs=2, name="ka")
                mm3(acc, wk_sb, wkr_sb, x8_t, rx8_t, msl, TB)
                nc.vector.tensor_scalar_mul(kt_sb[:, mt, tsl], acc[:], DS)
                yield
            for st in range(4):
                ssl = slice(st * P, (st + 1) * P)
                for half, fsl, w in ((0, slice(0, TB), TB),
                                     (1, slice(TB, D), D - TB)):
                    acc = ps.tile([P, TB], F32, tag="acc", bufs=2, name="va")
                    mm3t(acc, x8_t, rx8_t, ssl, wv_sb, wvr_sb, fsl, w)
                    src = acc[:, :w].rearrange("p (h e) -> p h e", e=64)
                    h0 = half * 8
                    dst = vd_view[:, tb * 4 + st, h0:h0 + w // 64, :]
                    nc.vector.tensor_scalar_mul(dst, src, DS)
                    yield
            if want_q:
                for mt in range(NC):
                    msl = slice(mt * P, (mt + 1) * P)
                    acc = ps.tile([P, TB], F32, tag="acc", bufs=2, name="qa")
                    mm3(acc, wq_sb, wqr_sb, x8_t, rx8_t, msl, TB)
                    nc.vector.tensor_scalar_mul(qt_sb[:, mt, tsl], acc[:], DS)
                    yield

        es_xtr = ExitStack()
        p_xtr = es_xtr.enter_context(tc.tile_pool(name="p_xtr", bufs=2))
        xtr_tiles = {}
        stats_res = {}

        def load_xt(tb, step=2):
            tsl = slice(tb * TB, (tb + 1) * TB)
            xt_t = p_xtr.tile([P, NC, TB], F32R, tag="xtr", name=f"xt_{tb}")
            xtr_tiles[tb] = xt_t
            for j0 in range(0, NC, step):
                nc.sync.dma_start(xt_t[:, j0:j0 + step, :],
                                  xt_r[:, j0:j0 + step, tsl])

        # A: tb0 stats
        _early_const_dmas()
        load_xt(0, step=1)
        for sb, d in ((wk_sb, wk8_d), (wkr_sb, wk8r_d), (wv_sb, wv8_d),
                      (wvr_sb, wv8r_d), (wq_sb, wq8_d), (wqr_sb, wq8r_d)):
            nc.sync.dma_start(sb[:], d.rearrange("(j p) m -> p j m", p=P))
        stats_res[0] = ln_stats(xtr_tiles[0][:])
        # B: tb2 stats; norm0 + qkv0
        load_xt(2)
        _late_const_dmas()
        stats_res[2] = ln_stats(xtr_tiles[2][:], sq_pool=True)
        for _ in norm_units(0):
            pass
        for _ in qkv_units(0, want_q=True):
            pass
        # C: tb1 stats; norm2 + qkv2
        load_xt(1)
        stats_res[1] = ln_stats(xtr_tiles[1][:], sq_pool=True)
        for _ in norm_units(2):
            pass
        for _ in qkv_units(2, want_q=False):
            pass
        # D: tb3 stats; norm1
        load_xt(3)
        stats_res[3] = ln_stats(xtr_tiles[3][:], sq_pool=True)
        for _ in norm_units(1):
            pass

        # ---------------- attention machinery ----------------

        def attn_units(sl_i):
            qsl = slice(sl_i * TB, (sl_i + 1) * TB)
            chunks = SLOT_CHUNKS[sl_i]
            last_ci = len(chunks) - 1
            for mt in range(NC):
                yt2 = [ps.tile([65, TB], F32, tag="yt", bufs=2,
                               name=f"yt{ph}") for ph in range(2)]

                def pv(ci, e_sb):
                    ch = chunks[ci]
                    for ph in range(2):
                        h = 2 * mt + ph
                        nc.tensor.matmul(
                            yt2[ph][:], v_sb[:, ch, h * 65:(h + 1) * 65],
                            e_sb[:, ph * TB:(ph + 1) * TB],
                            start=(ci == 0), stop=(ci == last_ci))

                pend = None
                for ci, ch in enumerate(chunks):
                    sb_idx = (0 if sl_i == 0 else 8) + ci
                    st2 = ps.tile([P, 2 * TB], F32, tag="st", bufs=2)
                    for ph in range(2):
                        o = ph * 64
                        nc.tensor.matmul(
                            st2[:, ph * TB:(ph + 1) * TB],
                            kt_sb[o:o + 64, mt, ch * P:(ch + 1) * P],
                            qt_sb[o:o + 64, mt, qsl], start=True, stop=True)
                    e_sb = p_e.tile([P, 2 * TB], BF16, tag="e")
                    nc.scalar.activation(
                        e_sb[:], st2[:], AF.Exp,
                        bias=bias_sb[:, sb_idx:sb_idx + 1],
                        scale=scale_sb[:, sb_idx:sb_idx + 1])
                    di = ch - 4 * sl_i
                    if 0 <= di < 4:
                        for ph in range(2):
                            nc.vector.tensor_mul(
                                e_sb[:, ph * TB:(ph + 1) * TB],
                                e_sb[:, ph * TB:(ph + 1) * TB],
                                masks_sb[:, di, :])
                    if pend is not None:
                        pv(*pend)
                    pend = (ci, e_sb)
                    yield
                pv(*pend)
                for ph in range(2):
                    o = ph * 64
                    rc = work.tile([1, TB], F32R, tag="rc")
                    nc.vector.reciprocal(rc[:], yt2[ph][64:65, :])
                    bc = ps.tile([64, TB], F32, tag="acc", bufs=2, name="bc")
                    nc.tensor.matmul(bc[:], onesbc_sb[0:1, 0:64], rc[:],
                                     start=True, stop=True)
                    nc.vector.tensor_copy(yt_all[o:o + 64, mt, :],
                                          yt2[ph][0:64, :])
                    nc.vector.tensor_mul(yt_all[o:o + 64, mt, :],
                                         yt_all[o:o + 64, mt, :], bc[:])
                yield

        def wo_ln2_units(sl_i):
            qsl = slice(sl_i * TB, (sl_i + 1) * TB)
            x28_sb, rx28_sb = S["xn2_sb"]
            wo_sb = S["wo_sb"]
            for ct in range(NC):
                ao = ps.tile([P, TB], F32, tag="acc", bufs=2, name="ao")
                for mc in range(NC):
                    nc.tensor.matmul(ao[:], wo_sb[:, mc, ct * P:(ct + 1) * P],
                                     yt_all[:, mc, :],
                                     start=(mc == 0), stop=(mc == NC - 1))
                nc.vector.tensor_add(xp_sb[:, ct, qsl],
                                     xp_sb[:, ct, qsl], ao[:])
                yield
            r2, mur2 = ln_stats(xp_sb[:, :, qsl])
            yield
            for j in range(NC):
                xnb = work.tile([P, TB], BF16, tag="xnb")
                ln_norm_chunk(xp_sb[:, j, qsl], xnb[:], r2, mur2,
                              g2_sb, be2_sb, j)
                split8_chunk(xnb, x28_sb[:, j, qsl], rx28_sb[:, j, qsl], j)
                yield

        # ---------------- MLP machinery ----------------
        w18_r = w18_d.rearrange("(j p) f -> p j f", p=P)
        w18r_r = w18r_d.rearrange("(j p) f -> p j f", p=P)
        w28_r = w28_d.rearrange("(f p) c -> p f c", p=P)
        w28r_r = w28r_d.rearrange("(f p) c -> p f c", p=P)
        outt_r = outt_d.rearrange("(j p) t -> p j t", p=P)

        def w1_units(sl_i):
            tsl = slice(sl_i * TB, (sl_i + 1) * TB)
            (h8_sb, rh8_sb), p_wmlp = S["h1_sb"], S["p_wmlp"]
            x28_sb, rx28_sb = S["xn2_sb"]

            def w1_dma(ft2):
                fs = slice(2 * ft2 * P, (2 * ft2 + 2) * P)
                w1_t = p_wmlp.tile([P, NC, 2 * P], E4, tag="w1",
                                   name=f"w1_{sl_i}_{ft2}")
                w1r_t = p_wmlp.tile([P, NC, 2 * P], E4, tag="w1r",
                                    name=f"w1r_{sl_i}_{ft2}")
                nc.sync.dma_start(w1_t[:], w18_r[:, :, fs])
                nc.sync.dma_start(w1r_t[:], w18r_r[:, :, fs])
                return w1_t, w1r_t
            nxt = w1_dma(0)
            for ft2 in range(NF // 2):
                w1_t, w1r_t = nxt
                if ft2 + 1 < NF // 2:
                    nxt = w1_dma(ft2 + 1)
                for sub in range(2):
                    ft = 2 * ft2 + sub
                    msl = slice(sub * P, (sub + 1) * P)
                    hp = ps.tile([P, TB], F32, tag="acc", bufs=2, name="hp")
                    mm3(hp, w1_t, w1r_t, x28_sb[:, :, tsl],
                        rx28_sb[:, :, tsl], msl, TB)
                    hb = work.tile([P, TB], F32R, tag="nrm1", name="hb")
                    nc.vector.tensor_scalar(hb[:], hp[:], DS,
                                            b1_sb[:, ft:ft + 1],
                                            ALU.mult, ALU.add)
                    hbf = work.tile([P, TB], BF16, tag="xnb", name="hbf")
                    nc.vector.tensor_scalar_max(hbf[:], hb[:], 0.0)
                    nc.vector.tensor_copy(h8_sb[:, ft, tsl], hbf[:])
                    nc.gpsimd.tensor_sub(rh8_sb[:, ft, tsl], hbf[:],
                                         h8_sb[:, ft, tsl])
                    yield

        w2_tiles = {}

        def w2_dma_units():
            p_w2 = S["p_w2"]
            for ft2 in range(NF // 2):
                fs = slice(2 * ft2, 2 * ft2 + 2)
                w2_t = p_w2.tile([P, 2, D], E4, tag=f"w2_{ft2}", bufs=1,
                                 name=f"w2_{ft2}")
                w2r_t = p_w2.tile([P, 2, D], E4, tag=f"w2r_{ft2}", bufs=1,
                                  name=f"w2r_{ft2}")
                nc.sync.dma_start(w2_t[:], w28_r[:, fs, :])
                nc.sync.dma_start(w2r_t[:], w28r_r[:, fs, :])
                w2_tiles[ft2] = (w2_t, w2r_t)
                yield

        def w2_units(g):
            (h8_sb, rh8_sb) = S["h1_sb"]
            p_out = S["p_out"]
            o2s = {}
            for ci, (t, bu) in enumerate((("acc", 2), ("yt", 2))):
                for tb in range(2):
                    o2s[(ci, tb)] = ps.tile([P, TB], F32, tag=t, bufs=bu,
                                            name=f"o2_{g}_{ci}_{tb}")
            stp = ps.tile([P, 2 * TB], F32, tag="st", bufs=2,
                          name=f"o2st_{g}")
            o2s[(2, 0)] = stp[:, 0:TB]
            o2s[(2, 1)] = stp[:, TB:2 * TB]

            for ft2 in range(NF // 2):
                w2_t, w2r_t = w2_tiles[ft2]
                for ci in range(3):
                    ct = g * 3 + ci
                    csl = slice(ct * P, (ct + 1) * P)
                    for tb in range(2):
                        hsl = slice(tb * TB, (tb + 1) * TB)
                        for wa, ha in ((w2_t, h8_sb), (w2_t, rh8_sb),
                                       (w2r_t, h8_sb)):
                            nc.tensor.matmul(
                                o2s[(ci, tb)][:], wa[:, :, csl],
                                ha[:, 2 * ft2:2 * ft2 + 2, hsl],
                                start=(ft2 == 0 and wa is w2_t
                                       and ha is h8_sb),
                                stop=(ft2 == NF // 2 - 1 and wa is w2r_t),
                                perf_mode=MM)
                yield
            for tb in range(2):
                tsl = slice(tb * TB, (tb + 1) * TB)
                for ci in range(3):
                    ct = g * 3 + ci
                    xpb = work.tile([P, TB], F32R, tag="nrm1", name="xpb")
                    nc.vector.tensor_scalar(xpb[:], xp_sb[:, ct, tsl],
                                            b2_sb[:, ct:ct + 1], None,
                                            ALU.add)
                    ot = p_out.tile([P, TB], F32, tag="ot")
                    nc.vector.scalar_tensor_tensor(
                        ot[:], o2s[(ci, tb)][:], DS, xpb[:],
                        ALU.mult, ALU.add)
                    nc.sync.dma_start(outt_r[:, ct, tsl], ot[:])
                    yield

        # ---------------- Phase E: slot0 attention woven with rest --------
        _mark(nc, "attn0")

        def e_fill():
            for u in norm_units(3):
                yield u
            es_xtr.close()
            for u in qkv_units(1, want_q=True):
                yield u
            for u in qkv_units(3, want_q=False):
                yield u
            es_xn1.close()
            es_wqkv.close()
            p_h1 = ctx.enter_context(tc.tile_pool(name="p_h1", bufs=1))
            S["h1_sb"] = (
                p_h1.tile([P, NF, 1024], E4, tag="h8", name="h8"),
                p_h1.tile([P, NF, 1024], E4, tag="rh8", name="rh8"))
            S["p_wmlp"] = ctx.enter_context(
                tc.tile_pool(name="p_wmlp", bufs=2))
            S["p_wo"] = es_wo = ExitStack()
            p_wo = es_wo.enter_context(tc.tile_pool(name="p_wo", bufs=1))
            S["wo_sb"] = wo_sb = p_wo.tile([P, NC, D], BF16, tag="wo", name="wo")
            nc.sync.dma_start(wo_sb[:],
                              wo_d.rearrange("(j p) m -> p j m", p=P))
            nc.sync.dma_start(xp_sb[:], xt_r[:, :, 0:1024])
            yield

        _weave(attn_units(0), e_fill(), fill_per=0.78)

        # ---------------- Phase F: slot0 wo + LN2 ----------------
        _mark(nc, "wo0")
        S["p_xn2"] = es_xn2 = ExitStack()
        p_xn2 = es_xn2.enter_context(tc.tile_pool(name="p_xn2", bufs=1))
        S["xn2_sb"] = (
            p_xn2.tile([P, NC, 1024], E4, tag="x28", name="x28"),
            p_xn2.tile([P, NC, 1024], E4, tag="rx28", name="rx28"))
        for _ in wo_ln2_units(0):
            pass

        # ---------------- Phase G: slot1 attention woven with W1(slot0) ---
        _mark(nc, "attn1")
        from itertools import islice
        w1g0 = w1_units(0)
        _weave(attn_units(1), islice(w1g0, 12), fill_per=0.125)

        # ---------------- Phase H: slot1 wo + LN2 woven with W1 rest ------
        _mark(nc, "wo1")
        _weave(wo_ln2_units(1), w1g0, fill_per=1.0)
        es_masks.close()
        es_kqv.close()
        _mark(nc, "mlp")
        S["p_w2"] = ctx.enter_context(
            tc.tile_pool(name="p_w2", bufs=1, side="right"))
        _weave(w1_units(1), w2_dma_units(), fill_per=1.0)
        S["p_xn2"].close()
        S["p_wo"].close()
        S["p_out"] = ctx.enter_context(tc.tile_pool(name="p_out", bufs=3))
        for _ in w2_units(0):
            pass
        for _ in w2_units(1):
            pass

    nc.compile()
    return nc


def _host_inputs(X, w_q, w_k, w_v, w_o, W1, b1, W2, b2, g1, be1, g2, be2):
    """Build the 8 per-core input dicts."""
    f32 = np.float32
    import ml_dtypes
    bf16 = ml_dtypes.bfloat16
    e4 = ml_dtypes.float8_e4m3

    def split8(w):
        ws = np.asarray(w, f32) * WS
        a = ws.astype(e4)
        r = (ws - a.astype(f32)).astype(e4)
        return np.ascontiguousarray(a), np.ascontiguousarray(r)

    wq8, wq8r = split8(np.asarray(w_q, f32).reshape(D, D).T)
    wk8, wk8r = split8(np.asarray(w_k, f32).reshape(D, D).T)
    wv8, wv8r = split8(np.asarray(w_v, f32).reshape(D, D).T)
    wo = np.ascontiguousarray(np.asarray(w_o, f32).astype(bf16))
    onesbc = np.ones((P, P), f32)
    # 4 canonical self-diagonal masks: mask[k][s, t] = (128k + s <= t)
    masks = np.zeros((4, P, TB), bf16)
    ar_s = np.arange(P)[:, None]
    ar_t = np.arange(TB)[None, :]
    for k in range(4):
        masks[k] = (128 * k + ar_s <= ar_t).astype(bf16)
    w18, w18r = split8(np.asarray(W1, f32).T)
    w28, w28r = split8(np.asarray(W2, f32).T)

    # per-role exp scale/bias: 24 = 8 (slot0) + 16 (slot1) chunk positions
    sc = {}
    bi = {}
    for role in range(2):
        order = ROLE_ORDER[role]
        s = np.full((24,), 0.125, f32)
        b = np.zeros((24,), f32)
        for sl_i in range(2):
            own_blk = order[sl_i]
            for ci, ch in enumerate(SLOT_CHUNKS[sl_i]):
                idx = (0 if sl_i == 0 else 8) + ci
                pos = ch // 4           # permuted 512-block of this s-chunk
                blk = order[pos]
                if pos == sl_i or blk < own_blk:
                    pass                # diagonal (tri-masked) or past: live
                else:
                    s[idx] = 0.0        # future: dead
                    b[idx] = DEAD
        sc[role] = np.broadcast_to(s, (P, 24)).copy()
        bi[role] = np.broadcast_to(b, (P, 24)).copy()

    shared = dict(wq8=wq8, wq8r=wq8r, wk8=wk8, wk8r=wk8r, wv8=wv8,
                  wv8r=wv8r, wo=wo, w18=w18, w18r=w18r, w28=w28, w28r=w28r,
                  onesbc=onesbc, masks=masks,
                  g1v=np.asarray(g1, f32), be1v=np.asarray(be1, f32),
                  g2v=np.asarray(g2, f32), be2v=np.asarray(be2, f32),
                  b1v=np.asarray(b1, f32), b2v=np.asarray(b2, f32))

    in_maps = []
    for core in range(8):
        role, b_idx = core // 4, core % 4
        order = ROLE_ORDER[role]
        xb = np.asarray(X[b_idx], f32)          # [T, D]
        xperm = np.concatenate([xb[o * TB:(o + 1) * TB] for o in order], axis=0)
        xt = np.ascontiguousarray(xperm.T)      # [D, T]
        m = dict(shared)
        m["xt"] = xt
        m["scalein"] = sc[role]
        m["biasin"] = bi[role]
        in_maps.append(m)
    return in_maps


def _assemble(results, dtype):
    out = np.empty((B, T, D), dtype)
    for core in range(8):
        role, b_idx = core // 4, core % 4
        order = ROLE_ORDER[role]
        ot = results[core]["outt"]              # [D, 1024]
        for sl_i in range(2):
            blk = order[sl_i]
            out[b_idx, blk * TB:(blk + 1) * TB] = \
                ot[:, sl_i * TB:(sl_i + 1) * TB].T
    return out


def kernel(X, w_q, w_k, w_v, w_o, W1, b1, W2, b2, g1, be1, g2, be2,
           _want_results=False, _trace=False):
    if "nc" not in _cached:
        _cached["nc"] = _build_nc()
    nc = _cached["nc"]
    in_maps = _host_inputs(X, w_q, w_k, w_v, w_o, W1, b1, W2, b2,
                           g1, be1, g2, be2)
    res = run_bass_kernel_spmd(nc, in_maps, core_ids=list(range(8)),
                               trace=_trace)
    out = _assemble(res.results, np.asarray(X).dtype)
    if _want_results:
        return out, res
    return out


# revision 24
# speedup vs baseline: 1.1561x; 1.0028x over previous
"""Trainium2 Bass kernel for a dense transformer block (pre-LN attn + MLP).

B=4, T=2048, D=768, H=12 (DH=64), DFF=3072, fp32.

Sharding: 8 cores = 4 batches x 2 roles. Each core processes one batch and
owns 1024 query tokens (two 512-blocks, paired {0,3}/{1,2} for causal load
balance). K/V are computed for the full 2048 tokens on both cores of a batch
(cheap), so there are NO collectives.

SPMD uniformity: all 8 cores run ONE identical NEFF. Causal structure is
carried in DATA, not code:
  - host permutes each batch's token axis to [own0, own1, otherA, otherB]
  - q-slot0 attends s-chunks {0..3, 8..11}; q-slot1 attends s-chunks {0..15}
  - per-(slot,chunk) exp scale/bias inputs select live / dead (zero) chunks
  - 4 canonical triangular masks handle the self-diagonal 512-blocks

Everything on-chip runs in a transposed layout (features on partitions,
tokens on the free axis). Matmul cost on the PE depends only on the output
free size, so per-token LayerNorm statistics are computed REPLICATED across
all 128 partitions (ones [128,128] lhsT) and softmax denominators come
replicated across 64 partitions from a shared ones-block appended to V —
no partition broadcasts are needed anywhere.

Engine assignment: PE does GEMMs + LN column-sum stats; Act does exp and the
LN apply (scale/bias copy); DVE does LN tensor ops, masks, softmax divides
and residuals; Pool (gpsimd) does all PSUM->SBUF copies, squares and relu.
Emission is software-pipelined: slot0 attention is woven with the remaining
QKV projections, slot1 attention with the first half of the MLP, so the PE
never idles behind the Act exp chain.
"""

import sys

sys.path.insert(0, "/opt/trn_rl_repo")

from contextlib import ExitStack

import numpy as np

import concourse.bass as bass
import concourse.mybir as mybir
import concourse.tile as tile
from concourse import bacc
from concourse.bass_utils import run_bass_kernel_spmd

F32 = mybir.dt.float32
E4 = mybir.dt.float8e4
F32R = mybir.dt.float32r
AF = mybir.ActivationFunctionType
BF16 = mybir.dt.bfloat16
ALU = mybir.AluOpType

H, D, DFF = 12, 768, 3072
DH = 64
B, T = 4, 2048
EPS = 1e-5
P = 128
NC = D // P          # 6 feature chunks
NF = DFF // P        # 24 ff tiles
TB = 512             # token block
NTB = T // TB        # 4 blocks
VW = H * 65          # 780: per head 64 values + 1 ones col
SLOT_CHUNKS = [[0, 1, 2, 3, 8, 9, 10, 11], list(range(16))]
# role -> permuted block order [own0, own1, restA, restB] (original block ids)
ROLE_ORDER = [[0, 3, 1, 2], [1, 2, 0, 3]]
DEAD = -30000.0      # exp(DEAD) == 0 in fp32
WS = 32.0            # fp8 weight scale; products carry WS, descaled at epilogue
DS = 1.0 / WS
MM = mybir.MatmulPerfMode.DoubleRow
TBORD = [0, 2, 1, 3]  # emission order: slot0 needs permuted blocks 0 and 2

_cached = {}
PHASE_MARKS = []


def _mark(nc, name):
    PHASE_MARKS.append((name, nc.next_id()))


def _weave(primary, filler, fill_per=1.0):
    """Drain primary generator; after each primary unit pull `fill_per`
    (fractional) units from filler. Leftover filler drains at the end."""
    acc = 0.0
    f = iter(filler)
    for _ in primary:
        acc += fill_per
        while acc >= 1.0:
            acc -= 1.0
            if next(f, StopIteration) is StopIteration:
                acc = 0.0
                break
    for _ in f:
        pass


def _build_nc():
    nc = bacc.Bacc("TRN2", target_bir_lowering=False, debug=False,
                   enable_asserts=False, num_devices=8)

    def din(name, shape, dt=F32R):
        return nc.dram_tensor(name, shape, dt, kind="ExternalInput").ap()

    xt_d = din("xt", [D, T])                 # X[b].T, token-permuted
    wq8_d = din("wq8", [D, D], E4)           # w_q as [c, m], x32
    wq8r_d = din("wq8r", [D, D], E4)         # residual, x32
    wk8_d = din("wk8", [D, D], E4)
    wk8r_d = din("wk8r", [D, D], E4)
    wv8_d = din("wv8", [D, D], E4)
    wv8r_d = din("wv8r", [D, D], E4)
    wo_d = din("wo", [D, D], BF16)           # natural [m, c]
    w18_d = din("w18", [D, DFF], E4)         # W1.T  [c, f], x32
    w18r_d = din("w18r", [D, DFF], E4)
    w28_d = din("w28", [DFF, D], E4)         # W2.T  [f, c], x32
    w28r_d = din("w28r", [DFF, D], E4)
    onesbc_d = din("onesbc", [P, P])         # all-ones lhsT: replicated sums
    masks_d = din("masks", [4, P, TB], BF16)  # tri masks
    scalein_d = din("scalein", [P, 24], F32) # exp scale per (slot,chunk)
    biasin_d = din("biasin", [P, 24], F32)   # exp bias per (slot,chunk)
    g1_d = din("g1v", [D], F32)
    be1_d = din("be1v", [D], F32)
    g2_d = din("g2v", [D], F32)
    be2_d = din("be2v", [D], F32)
    b1_d = din("b1v", [DFF], F32)
    b2_d = din("b2v", [D], F32)

    outt_d = nc.dram_tensor("outt", [D, 1024], F32, kind="ExternalOutput").ap()

    xt_r = xt_d.rearrange("(j p) t -> p j t", p=P)

    with tile.TileContext(nc) as tc, ExitStack() as ctx, \
         nc.allow_low_precision(reason="fp32r/bf16 intermediates are intended"):
        consts = ctx.enter_context(tc.tile_pool(name="consts", bufs=1))
        ps = ctx.enter_context(tc.tile_pool(name="ps", bufs=1, space="PSUM"))
        work = ctx.enter_context(tc.tile_pool(name="work", bufs=2))
        p_xp = ctx.enter_context(tc.tile_pool(name="p_xp", bufs=1))
        xp_sb = p_xp.tile([P, NC, 1024], F32R, tag="xp", name="xp")
        p_yt = ctx.enter_context(tc.tile_pool(name="p_yt", bufs=1))
        yt_all = p_yt.tile([P, NC, TB], BF16, tag="yt_all", name="yt_all")
        S = {"xp_sb": xp_sb}

        onesbc_sb = consts.tile([P, P], F32R, tag="onesbc")
        scale_sb = consts.tile([P, 24], F32, tag="scalein")
        bias_sb = consts.tile([P, 24], F32, tag="biasin")
        g1_sb = consts.tile([P, NC], F32, tag="g1")
        be1_sb = consts.tile([P, NC], F32, tag="be1")
        g2_sb = consts.tile([P, NC], F32, tag="g2")
        be2_sb = consts.tile([P, NC], F32, tag="be2")
        b1_sb = consts.tile([P, NF], F32, tag="b1")
        b2_sb = consts.tile([P, NC], F32, tag="b2")

        def _early_const_dmas():
            nc.sync.dma_start(onesbc_sb[:], onesbc_d)
            for sb, d in ((g1_sb, g1_d), (be1_sb, be1_d)):
                nc.sync.dma_start(sb[:], d.rearrange("(j p) -> p j", p=P))

        def _late_const_dmas():
            nc.sync.dma_start(scale_sb[:], scalein_d)
            nc.sync.dma_start(bias_sb[:], biasin_d)
            for sb, d in ((g2_sb, g2_d), (be2_sb, be2_d)):
                nc.sync.dma_start(sb[:], d.rearrange("(j p) -> p j", p=P))
            nc.sync.dma_start(b1_sb[:], b1_d.rearrange("(j p) -> p j", p=P))
            nc.sync.dma_start(b2_sb[:], b2_d.rearrange("(j p) -> p j", p=P))
            nc.sync.dma_start(masks_sb[:], masks_d.rearrange("o p t -> p o t"))

        # ---------------- LayerNorm helpers ----------------
        def ln_stats(src3, sq_pool=False):
            """src3: [128, NC, TB] slice. Returns (r, mur): [128, TB] tiles
            with 1/std and mu/std replicated across partitions."""
            s1 = ps.tile([P, TB], F32, tag="acc", bufs=2, name="s1")
            s2 = ps.tile([P, TB], F32, tag="acc", bufs=2, name="s2")
            for j in range(NC):
                nc.tensor.matmul(s1[:], onesbc_sb[:], src3[:, j, :],
                                 start=(j == 0), stop=(j == NC - 1))
            for j in range(NC):
                sq = work.tile([P, TB], F32R, tag="sq", bufs=2)
                if not sq_pool and j % 2 == 0:
                    nc.scalar.activation(sq[:], src3[:, j, :], AF.Square)
                else:
                    nc.gpsimd.tensor_mul(sq[:], src3[:, j, :], src3[:, j, :])
                nc.tensor.matmul(s2[:], onesbc_sb[:], sq[:],
                                 start=(j == 0), stop=(j == NC - 1))
            mu = work.tile([P, TB], F32, tag="mu", bufs=1)
            t = work.tile([P, TB], F32, tag="tmp", bufs=1)
            r = work.tile([P, TB], F32R, tag="r", bufs=2)
            mur = work.tile([P, TB], F32R, tag="mur", bufs=2)
            nc.vector.tensor_scalar_mul(mu[:], s1[:], 1.0 / D)
            nc.vector.tensor_mul(t[:], mu[:], mu[:])
            nc.vector.scalar_tensor_tensor(t[:], s2[:], 1.0 / D, t[:],
                                           ALU.mult, ALU.subtract)
            nc.vector.tensor_scalar_add(t[:], t[:], EPS)
            nc.scalar.activation(t[:], t[:], AF.Sqrt)
            nc.vector.reciprocal(r[:], t[:])
            nc.vector.tensor_mul(mur[:], mu[:], r[:])
            return r, mur

        def ln_norm_chunk(src_j, dst_j, r, mur, g_sb, be_sb, j):
            """dst = (src*r - mur)*g[p] + be[p]."""
            t1 = work.tile([P, TB], F32R, tag="nrm1")
            nc.vector.tensor_mul(t1[:], src_j, r[:])
            nc.vector.tensor_sub(t1[:], t1[:], mur[:])
            nc.scalar.activation(dst_j, t1[:], AF.Identity,
                                 bias=be_sb[:, j:j + 1], scale=g_sb[:, j:j + 1])

        # ---------------- attention-phase SBUF tensors ----------------
        es_kqv = ExitStack()
        p_kqv = es_kqv.enter_context(tc.tile_pool(name="p_kqv", bufs=1,
                                                  side="right"))
        kt_sb = p_kqv.tile([P, NC, T], BF16, tag="kt")      # K^T [m, s]
        qt_sb = p_kqv.tile([P, NC, 1024], BF16, tag="qt")   # Q^T [m, t_own]
        v_sb = p_kqv.tile([P, 16, VW], BF16, tag="v")       # V_ext [s, 780]
        v_hv = v_sb.rearrange("p s (h e) -> p s h e", e=65)
        vd_view = v_hv[:, :, :, 0:64]
        nc.vector.memset(v_hv[:, :, :, 64:65], 1.0)

        es_masks = ExitStack()
        p_masks = es_masks.enter_context(tc.tile_pool(name="p_masks", bufs=1,
                                                      side="right"))
        p_e = es_masks.enter_context(tc.tile_pool(name="p_e", bufs=2,
                                                  side="right"))
        masks_sb = p_masks.tile([P, 4, TB], BF16, tag="masks")

        es_wqkv = ExitStack()
        p_wqkv = es_wqkv.enter_context(tc.tile_pool(name="p_wqkv", bufs=1,
                                                    side="right"))
        wq_sb = p_wqkv.tile([P, NC, D], E4, tag="wq")
        wqr_sb = p_wqkv.tile([P, NC, D], E4, tag="wqr")
        wk_sb = p_wqkv.tile([P, NC, D], E4, tag="wk")
        wkr_sb = p_wqkv.tile([P, NC, D], E4, tag="wkr")
        wv_sb = p_wqkv.tile([P, NC, D], E4, tag="wv")
        wvr_sb = p_wqkv.tile([P, NC, D], E4, tag="wvr")

        # ---------------- Phase A-D: LN1 + QKV over TBORD, pipelined -------
        _mark(nc, "lnq")
        es_xn1 = ExitStack()
        p_xn1 = es_xn1.enter_context(tc.tile_pool(name="p_xn1", bufs=2))
        xn1_tiles = {}

        def split8_chunk(xnb, x8_j, rx8_j, j=0):
            nc.scalar.activation(x8_j, xnb[:], AF.Copy)
            if j % 2 == 0:
                nc.vector.tensor_sub(rx8_j, xnb[:], x8_j)
            else:
                nc.gpsimd.tensor_sub(rx8_j, xnb[:], x8_j)

        def norm_units(tb):
            r, mur = stats_res[tb]
            x8_t = p_xn1.tile([P, NC, TB], E4, tag="x8", name=f"x8_{tb}")
            rx8_t = p_xn1.tile([P, NC, TB], E4, tag="rx8", name=f"rx8_{tb}")
            xn1_tiles[tb] = (x8_t, rx8_t)
            xt_t = xtr_tiles[tb]
            for j in range(NC):
                xnb = work.tile([P, TB], BF16, tag="xnb")
                ln_norm_chunk(xt_t[:, j, :], xnb[:], r, mur,
                              g1_sb, be1_sb, j)
                split8_chunk(xnb, x8_t[:, j, :], rx8_t[:, j, :], j)
                yield

        def mm3(acc, wsb, wrsb, x8, rx8, msl, n):
            """acc += (w + rw).T (x + rx), 3-term fp8 DoubleRow over j pairs."""
            first = True
            for wa, xa in ((wsb, x8), (wsb, rx8), (wrsb, x8)):
                for jp in range(NC // 2):
                    j2 = slice(2 * jp, 2 * jp + 2)
                    nc.tensor.matmul(acc[:, :n], wa[:, j2, msl], xa[:, j2, :],
                                     start=first,
                                     stop=(wa is wrsb and jp == NC // 2 - 1),
                                     perf_mode=MM)
                    first = False

        def mm3t(acc, x8, rx8, ssl, wsb, wrsb, fsl, n):
            """acc += (x + rx).T (w + rw): x stationary variant."""
            first = True
            for xa, wa in ((x8, wsb), (x8, wrsb), (rx8, wsb)):
                for jp in range(NC // 2):
                    j2 = slice(2 * jp, 2 * jp + 2)
                    nc.tensor.matmul(acc[:, :n], xa[:, j2, ssl], wa[:, j2, fsl],
                                     start=first,
                                     stop=(xa is rx8 and jp == NC // 2 - 1),
                                     perf_mode=MM)
                    first = False

        def qkv_units(tb, want_q):
            tsl = slice(tb * TB, (tb + 1) * TB)
            x8_t, rx8_t = xn1_tiles[tb]
            for mt in range(NC):
                msl = slice(mt * P, (mt + 1) * P)
                acc = ps.tile([P, TB], F32, tag="acc", bufs=2, name="ka")
                mm3(acc, wk_sb, wkr_sb, x8_t, rx8_t, msl, TB)
                nc.vector.tensor_scalar_mul(kt_sb[:, mt, tsl], acc[:], DS)
                yield
            for st in range(4):
                ssl = slice(st * P, (st + 1) * P)
                for half, fsl, w in ((0, slice(0, TB), TB),
                                     (1, slice(TB, D), D - TB)):
                    acc = ps.tile([P, TB], F32, tag="acc", bufs=2, name="va")
                    mm3t(acc, x8_t, rx8_t, ssl, wv_sb, wvr_sb, fsl, w)
                    src = acc[:, :w].rearrange("p (h e) -> p h e", e=64)
                    h0 = half * 8
                    dst = vd_view[:, tb * 4 + st, h0:h0 + w // 64, :]
                    nc.vector.tensor_scalar_mul(dst, src, DS)
                    yield
            if want_q:
                for mt in range(NC):
                    msl = slice(mt * P, (mt + 1) * P)
                    acc = ps.tile([P, TB], F32, tag="acc", bufs=2, name="qa")
                    mm3(acc, wq_sb, wqr_sb, x8_t, rx8_t, msl, TB)
                    nc.vector.tensor_scalar_mul(qt_sb[:, mt, tsl], acc[:], DS)
                    yield

        es_xtr = ExitStack()
        p_xtr = es_xtr.enter_context(tc.tile_pool(name="p_xtr", bufs=2))
        xtr_tiles = {}
        stats_res = {}

        def load_xt(tb, step=2):
            tsl = slice(tb * TB, (tb + 1) * TB)
            xt_t = p_xtr.tile([P, NC, TB], F32R, tag="xtr", name=f"xt_{tb}")
            xtr_tiles[tb] = xt_t
            for j0 in range(0, NC, step):
                nc.sync.dma_start(xt_t[:, j0:j0 + step, :],
                                  xt_r[:, j0:j0 + step, tsl])

        # A: tb0 stats
        _early_const_dmas()
        load_xt(0, step=1)
        for sb, d in ((wk_sb, wk8_d), (wkr_sb, wk8r_d), (wv_sb, wv8_d),
                      (wvr_sb, wv8r_d), (wq_sb, wq8_d), (wqr_sb, wq8r_d)):
            nc.sync.dma_start(sb[:], d.rearrange("(j p) m -> p j m", p=P))
        stats_res[0] = ln_stats(xtr_tiles[0][:], sq_pool=True)
        # B: tb2 stats; norm0 + qkv0
        load_xt(2)
        _late_const_dmas()
        stats_res[2] = ln_stats(xtr_tiles[2][:], sq_pool=True)
        for _ in norm_units(0):
            pass
        for _ in qkv_units(0, want_q=True):
            pass
        # C: tb1 stats; norm2 + qkv2
        load_xt(1)
        stats_res[1] = ln_stats(xtr_tiles[1][:], sq_pool=True)
        for _ in norm_units(2):
            pass
        for _ in qkv_units(2, want_q=False):
            pass
        # D: tb3 stats; norm1
        load_xt(3)
        stats_res[3] = ln_stats(xtr_tiles[3][:], sq_pool=True)
        for _ in norm_units(1):
            pass

        # ---------------- attention machinery ----------------

        def attn_units(sl_i):
            qsl = slice(sl_i * TB, (sl_i + 1) * TB)
            chunks = SLOT_CHUNKS[sl_i]
            last_ci = len(chunks) - 1
            for mt in range(NC):
                yt2 = [ps.tile([65, TB], F32, tag="yt", bufs=2,
                               name=f"yt{ph}") for ph in range(2)]

                def pv(ci, e_sb):
                    ch = chunks[ci]
                    for ph in range(2):
                        h = 2 * mt + ph
                        nc.tensor.matmul(
                            yt2[ph][:], v_sb[:, ch, h * 65:(h + 1) * 65],
                            e_sb[:, ph * TB:(ph + 1) * TB],
                            start=(ci == 0), stop=(ci == last_ci))

                pend = None
                for ci, ch in enumerate(chunks):
                    sb_idx = (0 if sl_i == 0 else 8) + ci
                    st2 = ps.tile([P, 2 * TB], F32, tag="st", bufs=2)
                    for ph in range(2):
                        o = ph * 64
                        nc.tensor.matmul(
                            st2[:, ph * TB:(ph + 1) * TB],
                            kt_sb[o:o + 64, mt, ch * P:(ch + 1) * P],
                            qt_sb[o:o + 64, mt, qsl], start=True, stop=True)
                    e_sb = p_e.tile([P, 2 * TB], BF16, tag="e")
                    nc.scalar.activation(
                        e_sb[:], st2[:], AF.Exp,
                        bias=bias_sb[:, sb_idx:sb_idx + 1],
                        scale=scale_sb[:, sb_idx:sb_idx + 1])
                    di = ch - 4 * sl_i
                    if 0 <= di < 4:
                        for ph in range(2):
                            nc.vector.tensor_mul(
                                e_sb[:, ph * TB:(ph + 1) * TB],
                                e_sb[:, ph * TB:(ph + 1) * TB],
                                masks_sb[:, di, :])
                    if pend is not None:
                        pv(*pend)
                    pend = (ci, e_sb)
                    yield
                pv(*pend)
                for ph in range(2):
                    o = ph * 64
                    rc = work.tile([1, TB], F32R, tag="rc")
                    nc.vector.reciprocal(rc[:], yt2[ph][64:65, :])
                    bc = ps.tile([64, TB], F32, tag="acc", bufs=2, name="bc")
                    nc.tensor.matmul(bc[:], onesbc_sb[0:1, 0:64], rc[:],
                                     start=True, stop=True)
                    nc.vector.tensor_copy(yt_all[o:o + 64, mt, :],
                                          yt2[ph][0:64, :])
                    nc.vector.tensor_mul(yt_all[o:o + 64, mt, :],
                                         yt_all[o:o + 64, mt, :], bc[:])
                yield

        def wo_ln2_units(sl_i):
            qsl = slice(sl_i * TB, (sl_i + 1) * TB)
            x28_sb, rx28_sb = S["xn2_sb"]
            wo_sb = S["wo_sb"]
            for ct in range(NC):
                ao = ps.tile([P, TB], F32, tag="acc", bufs=2, name="ao")
                for mc in range(NC):
                    nc.tensor.matmul(ao[:], wo_sb[:, mc, ct * P:(ct + 1) * P],
                                     yt_all[:, mc, :],
                                     start=(mc == 0), stop=(mc == NC - 1))
                nc.vector.tensor_add(xp_sb[:, ct, qsl],
                                     xp_sb[:, ct, qsl], ao[:])
                yield
            r2, mur2 = ln_stats(xp_sb[:, :, qsl])
            yield
            for j in range(NC):
                xnb = work.tile([P, TB], BF16, tag="xnb")
                ln_norm_chunk(xp_sb[:, j, qsl], xnb[:], r2, mur2,
                              g2_sb, be2_sb, j)
                split8_chunk(xnb, x28_sb[:, j, qsl], rx28_sb[:, j, qsl], j)
                yield

        # ---------------- MLP machinery ----------------
        w18_r = w18_d.rearrange("(j p) f -> p j f", p=P)
        w18r_r = w18r_d.rearrange("(j p) f -> p j f", p=P)
        w28_r = w28_d.rearrange("(f p) c -> p f c", p=P)
        w28r_r = w28r_d.rearrange("(f p) c -> p f c", p=P)
        outt_r = outt_d.rearrange("(j p) t -> p j t", p=P)

        def w1_units(sl_i):
            tsl = slice(sl_i * TB, (sl_i + 1) * TB)
            (h8_sb, rh8_sb), p_wmlp = S["h1_sb"], S["p_wmlp"]
            x28_sb, rx28_sb = S["xn2_sb"]

            def w1_dma(ft2):
                fs = slice(2 * ft2 * P, (2 * ft2 + 2) * P)
                w1_t = p_wmlp.tile([P, NC, 2 * P], E4, tag="w1", bufs=3,
                                   name=f"w1_{sl_i}_{ft2}")
                w1r_t = p_wmlp.tile([P, NC, 2 * P], E4, tag="w1r", bufs=3,
                                    name=f"w1r_{sl_i}_{ft2}")
                nc.sync.dma_start(w1_t[:], w18_r[:, :, fs])
                nc.sync.dma_start(w1r_t[:], w18r_r[:, :, fs])
                return w1_t, w1r_t
            nxt = w1_dma(0)
            for ft2 in range(NF // 2):
                w1_t, w1r_t = nxt
                if ft2 + 1 < NF // 2:
                    nxt = w1_dma(ft2 + 1)
                for sub in range(2):
                    ft = 2 * ft2 + sub
                    msl = slice(sub * P, (sub + 1) * P)
                    hp = ps.tile([P, TB], F32, tag="acc", bufs=2, name="hp")
                    mm3(hp, w1_t, w1r_t, x28_sb[:, :, tsl],
                        rx28_sb[:, :, tsl], msl, TB)
                    hb = work.tile([P, TB], F32R, tag="nrm1", name="hb")
                    nc.vector.tensor_scalar(hb[:], hp[:], DS,
                                            b1_sb[:, ft:ft + 1],
                                            ALU.mult, ALU.add)
                    hbf = work.tile([P, TB], BF16, tag="xnb", name="hbf")
                    nc.vector.tensor_scalar_max(hbf[:], hb[:], 0.0)
                    nc.vector.tensor_copy(h8_sb[:, ft, tsl], hbf[:])
                    nc.gpsimd.tensor_sub(rh8_sb[:, ft, tsl], hbf[:],
                                         h8_sb[:, ft, tsl])
                    yield

        w2_tiles = {}

        def w2_dma_units():
            p_w2 = S["p_w2"]
            for ft2 in range(NF // 2):
                fs = slice(2 * ft2, 2 * ft2 + 2)
                w2_t = p_w2.tile([P, 2, D], E4, tag=f"w2_{ft2}", bufs=1,
                                 name=f"w2_{ft2}")
                w2r_t = p_w2.tile([P, 2, D], E4, tag=f"w2r_{ft2}", bufs=1,
                                  name=f"w2r_{ft2}")
                nc.sync.dma_start(w2_t[:], w28_r[:, fs, :])
                nc.sync.dma_start(w2r_t[:], w28r_r[:, fs, :])
                w2_tiles[ft2] = (w2_t, w2r_t)
                yield

        def w2_units(g):
            (h8_sb, rh8_sb) = S["h1_sb"]
            p_out = S["p_out"]
            o2s = {}
            for ci, (t, bu) in enumerate((("acc", 2), ("yt", 2))):
                for tb in range(2):
                    o2s[(ci, tb)] = ps.tile([P, TB], F32, tag=t, bufs=bu,
                                            name=f"o2_{g}_{ci}_{tb}")
            stp = ps.tile([P, 2 * TB], F32, tag="st", bufs=2,
                          name=f"o2st_{g}")
            o2s[(2, 0)] = stp[:, 0:TB]
            o2s[(2, 1)] = stp[:, TB:2 * TB]

            for ft2 in range(NF // 2):
                w2_t, w2r_t = w2_tiles[ft2]
                for ci in range(3):
                    ct = g * 3 + ci
                    csl = slice(ct * P, (ct + 1) * P)
                    for tb in range(2):
                        hsl = slice(tb * TB, (tb + 1) * TB)
                        for wa, ha in ((w2_t, h8_sb), (w2_t, rh8_sb),
                                       (w2r_t, h8_sb)):
                            nc.tensor.matmul(
                                o2s[(ci, tb)][:], wa[:, :, csl],
                                ha[:, 2 * ft2:2 * ft2 + 2, hsl],
                                start=(ft2 == 0 and wa is w2_t
                                       and ha is h8_sb),
                                stop=(ft2 == NF // 2 - 1 and wa is w2r_t),
                                perf_mode=MM)
                yield
            for tb in range(2):
                tsl = slice(tb * TB, (tb + 1) * TB)
                for ci in range(3):
                    ct = g * 3 + ci
                    xpb = work.tile([P, TB], F32R, tag="nrm1", name="xpb")
                    nc.vector.tensor_scalar(xpb[:], xp_sb[:, ct, tsl],
                                            b2_sb[:, ct:ct + 1], None,
                                            ALU.add)
                    ot = p_out.tile([P, TB], F32, tag="ot")
                    nc.vector.scalar_tensor_tensor(
                        ot[:], o2s[(ci, tb)][:], DS, xpb[:],
                        ALU.mult, ALU.add)
                    nc.sync.dma_start(outt_r[:, ct, tsl], ot[:])
                    yield

        # ---------------- Phase E: slot0 attention woven with rest --------
        _mark(nc, "attn0")

        def e_fill():
            for u in norm_units(3):
                yield u
            es_xtr.close()
            for u in qkv_units(1, want_q=True):
                yield u
            for u in qkv_units(3, want_q=False):
                yield u
            es_xn1.close()
            es_wqkv.close()
            p_h1 = ctx.enter_context(tc.tile_pool(name="p_h1", bufs=1))
            S["h1_sb"] = (
                p_h1.tile([P, NF, 1024], E4, tag="h8", name="h8"),
                p_h1.tile([P, NF, 1024], E4, tag="rh8", name="rh8"))
            S["p_wmlp"] = ctx.enter_context(
                tc.tile_pool(name="p_wmlp", bufs=2))
            S["p_wo"] = es_wo = ExitStack()
            p_wo = es_wo.enter_context(tc.tile_pool(name="p_wo", bufs=1))
            S["wo_sb"] = wo_sb = p_wo.tile([P, NC, D], BF16, tag="wo", name="wo")
            nc.sync.dma_start(wo_sb[:],
                              wo_d.rearrange("(j p) m -> p j m", p=P))
            nc.sync.dma_start(xp_sb[:], xt_r[:, :, 0:1024])
            yield

        _weave(attn_units(0), e_fill(), fill_per=0.78)

        # ---------------- Phase F: slot0 wo + LN2 ----------------
        _mark(nc, "wo0")
        S["p_xn2"] = es_xn2 = ExitStack()
        p_xn2 = es_xn2.enter_context(tc.tile_pool(name="p_xn2", bufs=1))
        S["xn2_sb"] = (
            p_xn2.tile([P, NC, 1024], E4, tag="x28", name="x28"),
            p_xn2.tile([P, NC, 1024], E4, tag="rx28", name="rx28"))
        for _ in wo_ln2_units(0):
            pass

        # ---------------- Phase G: slot1 attention woven with W1(slot0) ---
        _mark(nc, "attn1")
        from itertools import islice
        w1g0 = w1_units(0)
        _weave(attn_units(1), islice(w1g0, 12), fill_per=0.125)

        # ---------------- Phase H: slot1 wo + LN2 woven with W1 rest ------
        _mark(nc, "wo1")
        _weave(wo_ln2_units(1), w1g0, fill_per=1.0)
        es_masks.close()
        es_kqv.close()
        _mark(nc, "mlp")
        S["p_w2"] = ctx.enter_context(
            tc.tile_pool(name="p_w2", bufs=1, side="right"))
        _weave(w1_units(1), w2_dma_units(), fill_per=1.0)
        S["p_xn2"].close()
        S["p_wo"].close()
        S["p_out"] = ctx.enter_context(tc.tile_pool(name="p_out", bufs=3))
        for _ in w2_units(0):
            pass
        for _ in w2_units(1):
            pass

    nc.compile()
    return nc


def _host_inputs(X, w_q, w_k, w_v, w_o, W1, b1, W2, b2, g1, be1, g2, be2):
    """Build the 8 per-core input dicts."""
    f32 = np.float32
    import ml_dtypes
    bf16 = ml_dtypes.bfloat16
    e4 = ml_dtypes.float8_e4m3

    def split8(w):
        ws = np.asarray(w, f32) * WS
        a = ws.astype(e4)
        r = (ws - a.astype(f32)).astype(e4)
        return np.ascontiguousarray(a), np.ascontiguousarray(r)

    wq8, wq8r = split8(np.asarray(w_q, f32).reshape(D, D).T)
    wk8, wk8r = split8(np.asarray(w_k, f32).reshape(D, D).T)
    wv8, wv8r = split8(np.asarray(w_v, f32).reshape(D, D).T)
    wo = np.ascontiguousarray(np.asarray(w_o, f32).astype(bf16))
    onesbc = np.ones((P, P), f32)
    # 4 canonical self-diagonal masks: mask[k][s, t] = (128k + s <= t)
    masks = np.zeros((4, P, TB), bf16)
    ar_s = np.arange(P)[:, None]
    ar_t = np.arange(TB)[None, :]
    for k in range(4):
        masks[k] = (128 * k + ar_s <= ar_t).astype(bf16)
    w18, w18r = split8(np.asarray(W1, f32).T)
    w28, w28r = split8(np.asarray(W2, f32).T)

    # per-role exp scale/bias: 24 = 8 (slot0) + 16 (slot1) chunk positions
    sc = {}
    bi = {}
    for role in range(2):
        order = ROLE_ORDER[role]
        s = np.full((24,), 0.125, f32)
        b = np.zeros((24,), f32)
        for sl_i in range(2):
            own_blk = order[sl_i]
            for ci, ch in enumerate(SLOT_CHUNKS[sl_i]):
                idx = (0 if sl_i == 0 else 8) + ci
                pos = ch // 4           # permuted 512-block of this s-chunk
                blk = order[pos]
                if pos == sl_i or blk < own_blk:
                    pass                # diagonal (tri-masked) or past: live
                else:
                    s[idx] = 0.0        # future: dead
                    b[idx] = DEAD
        sc[role] = np.broadcast_to(s, (P, 24)).copy()
        bi[role] = np.broadcast_to(b, (P, 24)).copy()

    shared = dict(wq8=wq8, wq8r=wq8r, wk8=wk8, wk8r=wk8r, wv8=wv8,
                  wv8r=wv8r, wo=wo, w18=w18, w18r=w18r, w28=w28, w28r=w28r,
                  onesbc=onesbc, masks=masks,
                  g1v=np.asarray(g1, f32), be1v=np.asarray(be1, f32),
                  g2v=np.asarray(g2, f32), be2v=np.asarray(be2, f32),
                  b1v=np.asarray(b1, f32), b2v=np.asarray(b2, f32))

    in_maps = []
    for core in range(8):
        role, b_idx = core // 4, core % 4
        order = ROLE_ORDER[role]
        xb = np.asarray(X[b_idx], f32)          # [T, D]
        xperm = np.concatenate([xb[o * TB:(o + 1) * TB] for o in order], axis=0)
        xt = np.ascontiguousarray(xperm.T)      # [D, T]
        m = dict(shared)
        m["xt"] = xt
        m["scalein"] = sc[role]
        m["biasin"] = bi[role]
        in_maps.append(m)
    return in_maps


def _assemble(results, dtype):
    out = np.empty((B, T, D), dtype)
    for core in range(8):
        role, b_idx = core // 4, core % 4
        order = ROLE_ORDER[role]
        ot = results[core]["outt"]              # [D, 1024]
        for sl_i in range(2):
            blk = order[sl_i]
            out[b_idx, blk * TB:(blk + 1) * TB] = \
                ot[:, sl_i * TB:(sl_i + 1) * TB].T
    return out


def kernel(X, w_q, w_k, w_v, w_o, W1, b1, W2, b2, g1, be1, g2, be2,
           _want_results=False, _trace=False):
    if "nc" not in _cached:
        _cached["nc"] = _build_nc()
    nc = _cached["nc"]
    in_maps = _host_inputs(X, w_q, w_k, w_v, w_o, W1, b1, W2, b2,
                           g1, be1, g2, be2)
    res = run_bass_kernel_spmd(nc, in_maps, core_ids=list(range(8)),
                               trace=_trace)
    out = _assemble(res.results, np.asarray(X).dtype)
    if _want_results:
        return out, res
    return out


# revision 28
# speedup vs baseline: 1.1736x; 1.0152x over previous
"""Trainium2 Bass kernel for a dense transformer block (pre-LN attn + MLP).

B=4, T=2048, D=768, H=12 (DH=64), DFF=3072, fp32.

Sharding: 8 cores = 4 batches x 2 roles. Each core processes one batch and
owns 1024 query tokens (two 512-blocks, paired {0,3}/{1,2} for causal load
balance). K/V are computed for the full 2048 tokens on both cores of a batch
(cheap), so there are NO collectives.

SPMD uniformity: all 8 cores run ONE identical NEFF. Causal structure is
carried in DATA, not code:
  - host permutes each batch's token axis to [own0, own1, otherA, otherB]
  - q-slot0 attends s-chunks {0..3, 8..11}; q-slot1 attends s-chunks {0..15}
  - per-(slot,chunk) exp scale/bias inputs select live / dead (zero) chunks
  - 4 canonical triangular masks handle the self-diagonal 512-blocks

Everything on-chip runs in a transposed layout (features on partitions,
tokens on the free axis). Matmul cost on the PE depends only on the output
free size, so per-token LayerNorm statistics are computed REPLICATED across
all 128 partitions (ones [128,128] lhsT) and softmax denominators come
replicated across 64 partitions from a shared ones-block appended to V —
no partition broadcasts are needed anywhere.

Engine assignment: PE does GEMMs + LN column-sum stats; Act does exp and the
LN apply (scale/bias copy); DVE does LN tensor ops, masks, softmax divides
and residuals; Pool (gpsimd) does all PSUM->SBUF copies, squares and relu.
Emission is software-pipelined: slot0 attention is woven with the remaining
QKV projections, slot1 attention with the first half of the MLP, so the PE
never idles behind the Act exp chain.
"""

import sys

sys.path.insert(0, "/opt/trn_rl_repo")

from contextlib import ExitStack

import numpy as np

import concourse.bass as bass
import concourse.mybir as mybir
import concourse.tile as tile
from concourse import bacc
from concourse.bass_utils import run_bass_kernel_spmd

F32 = mybir.dt.float32
E4 = mybir.dt.float8e4
F32R = mybir.dt.float32r
AF = mybir.ActivationFunctionType
BF16 = mybir.dt.bfloat16
ALU = mybir.AluOpType

H, D, DFF = 12, 768, 3072
DH = 64
B, T = 4, 2048
EPS = 1e-5
P = 128
NC = D // P          # 6 feature chunks
NF = DFF // P        # 24 ff tiles
TB = 512             # token block
NTB = T // TB        # 4 blocks
VW = H * 65          # 780: per head 64 values + 1 ones col
SLOT_CHUNKS = [[0, 1, 2, 3, 8, 9, 10, 11], list(range(16))]
# role -> permuted block order [own0, own1, restA, restB] (original block ids)
ROLE_ORDER = [[0, 3, 1, 2], [1, 2, 0, 3]]
DEAD = -30000.0      # exp(DEAD) == 0 in fp32
WS = 32.0            # fp8 weight scale; products carry WS, descaled at epilogue
DS = 1.0 / WS
MM = mybir.MatmulPerfMode.DoubleRow
TBORD = [0, 2, 1, 3]  # emission order: slot0 needs permuted blocks 0 and 2

_cached = {}
PHASE_MARKS = []


def _mark(nc, name):
    PHASE_MARKS.append((name, nc.next_id()))


def _weave(primary, filler, fill_per=1.0):
    """Drain primary generator; after each primary unit pull `fill_per`
    (fractional) units from filler. Leftover filler drains at the end."""
    acc = 0.0
    f = iter(filler)
    for _ in primary:
        acc += fill_per
        while acc >= 1.0:
            acc -= 1.0
            if next(f, StopIteration) is StopIteration:
                acc = 0.0
                break
    for _ in f:
        pass


def _build_nc():
    nc = bacc.Bacc("TRN2", target_bir_lowering=False, debug=False,
                   enable_asserts=False, num_devices=8)

    def din(name, shape, dt=F32R):
        return nc.dram_tensor(name, shape, dt, kind="ExternalInput").ap()

    xt_d = din("xt", [D, T])                 # X[b].T, token-permuted
    wq8_d = din("wq8", [D, D], E4)           # w_q as [c, m], x32
    wq8r_d = din("wq8r", [D, D], E4)         # residual, x32
    wk8_d = din("wk8", [D, D], E4)
    wk8r_d = din("wk8r", [D, D], E4)
    wv8_d = din("wv8", [D, D], E4)
    wv8r_d = din("wv8r", [D, D], E4)
    wo_d = din("wo", [D, D], BF16)           # natural [m, c]
    w18_d = din("w18", [D, DFF], E4)         # W1.T  [c, f], x32
    w18r_d = din("w18r", [D, DFF], E4)
    w28_d = din("w28", [DFF, D], E4)         # W2.T  [f, c], x32
    w28r_d = din("w28r", [DFF, D], E4)
    onesbc_d = din("onesbc", [P, P])         # all-ones lhsT: replicated sums
    masks_d = din("masks", [4, P, TB], BF16)  # tri masks
    scalein_d = din("scalein", [P, 24], F32) # exp scale per (slot,chunk)
    biasin_d = din("biasin", [P, 24], F32)   # exp bias per (slot,chunk)
    g1_d = din("g1v", [D], F32)
    be1_d = din("be1v", [D], F32)
    g2_d = din("g2v", [D], F32)
    be2_d = din("be2v", [D], F32)
    b1_d = din("b1v", [DFF], F32)
    b2_d = din("b2v", [D], F32)

    outt_d = nc.dram_tensor("outt", [D, 1024], F32, kind="ExternalOutput").ap()

    xt_r = xt_d.rearrange("(j p) t -> p j t", p=P)

    with tile.TileContext(nc) as tc, ExitStack() as ctx, \
         nc.allow_low_precision(reason="fp32r/bf16 intermediates are intended"):
        consts = ctx.enter_context(tc.tile_pool(name="consts", bufs=1))
        ps = ctx.enter_context(tc.tile_pool(name="ps", bufs=1, space="PSUM"))
        work = ctx.enter_context(tc.tile_pool(name="work", bufs=2))
        p_xp = ctx.enter_context(tc.tile_pool(name="p_xp", bufs=1))
        xp_sb = p_xp.tile([P, NC, 1024], F32R, tag="xp", name="xp")
        p_yt = ctx.enter_context(tc.tile_pool(name="p_yt", bufs=1))
        yt_all = p_yt.tile([P, NC, TB], BF16, tag="yt_all", name="yt_all")
        S = {"xp_sb": xp_sb}

        onesbc_sb = consts.tile([P, P], F32R, tag="onesbc")
        scale_sb = consts.tile([P, 24], F32, tag="scalein")
        bias_sb = consts.tile([P, 24], F32, tag="biasin")
        g1_sb = consts.tile([P, NC], F32, tag="g1")
        be1_sb = consts.tile([P, NC], F32, tag="be1")
        g2_sb = consts.tile([P, NC], F32, tag="g2")
        be2_sb = consts.tile([P, NC], F32, tag="be2")
        b1_sb = consts.tile([P, NF], F32, tag="b1")
        b2_sb = consts.tile([P, NC], F32, tag="b2")

        def _early_const_dmas():
            nc.sync.dma_start(onesbc_sb[:], onesbc_d)
            for sb, d in ((g1_sb, g1_d), (be1_sb, be1_d)):
                nc.sync.dma_start(sb[:], d.rearrange("(j p) -> p j", p=P))

        def _late_const_dmas():
            nc.sync.dma_start(scale_sb[:], scalein_d)
            nc.sync.dma_start(bias_sb[:], biasin_d)
            for sb, d in ((g2_sb, g2_d), (be2_sb, be2_d)):
                nc.sync.dma_start(sb[:], d.rearrange("(j p) -> p j", p=P))
            nc.sync.dma_start(b1_sb[:], b1_d.rearrange("(j p) -> p j", p=P))
            nc.sync.dma_start(b2_sb[:], b2_d.rearrange("(j p) -> p j", p=P))
            nc.sync.dma_start(masks_sb[:], masks_d.rearrange("o p t -> p o t"))

        # ---------------- LayerNorm helpers ----------------
        def ln_stats(src3, sq_pool=False):
            """src3: [128, NC, TB] slice. Returns (r, mur): [128, TB] tiles
            with 1/std and mu/std replicated across partitions."""
            s1 = ps.tile([P, TB], F32, tag="acc", bufs=2, name="s1")
            s2 = ps.tile([P, TB], F32, tag="acc", bufs=2, name="s2")
            for j in range(NC):
                nc.tensor.matmul(s1[:], onesbc_sb[:], src3[:, j, :],
                                 start=(j == 0), stop=(j == NC - 1))
            for j in range(NC):
                sq = work.tile([P, TB], F32R, tag="sq", bufs=2)
                if not sq_pool and j % 2 == 0:
                    nc.scalar.activation(sq[:], src3[:, j, :], AF.Square)
                else:
                    nc.gpsimd.tensor_mul(sq[:], src3[:, j, :], src3[:, j, :])
                nc.tensor.matmul(s2[:], onesbc_sb[:], sq[:],
                                 start=(j == 0), stop=(j == NC - 1))
            mu = work.tile([P, TB], F32, tag="mu", bufs=1)
            t = work.tile([P, TB], F32, tag="tmp", bufs=1)
            r = work.tile([P, TB], F32R, tag="r", bufs=2)
            mur = work.tile([P, TB], F32R, tag="mur", bufs=2)
            nc.vector.tensor_scalar_mul(mu[:], s1[:], 1.0 / D)
            nc.vector.tensor_mul(t[:], mu[:], mu[:])
            nc.vector.scalar_tensor_tensor(t[:], s2[:], 1.0 / D, t[:],
                                           ALU.mult, ALU.subtract)
            nc.vector.tensor_scalar_add(t[:], t[:], EPS)
            nc.scalar.activation(t[:], t[:], AF.Sqrt)
            nc.vector.reciprocal(r[:], t[:])
            nc.vector.tensor_mul(mur[:], mu[:], r[:])
            return r, mur

        def ln_norm_chunk(src_j, dst_j, r, mur, g_sb, be_sb, j):
            """dst = (src*r - mur)*g[p] + be[p]."""
            t1 = work.tile([P, TB], F32R, tag="nrm1")
            nc.vector.tensor_mul(t1[:], src_j, r[:])
            nc.vector.tensor_sub(t1[:], t1[:], mur[:])
            nc.scalar.activation(dst_j, t1[:], AF.Identity,
                                 bias=be_sb[:, j:j + 1], scale=g_sb[:, j:j + 1])

        # ---------------- attention-phase SBUF tensors ----------------
        es_kqv = ExitStack()
        p_kqv = es_kqv.enter_context(tc.tile_pool(name="p_kqv", bufs=1,
                                                  side="right"))
        kt_sb = p_kqv.tile([P, NC, T], BF16, tag="kt")      # K^T [m, s]
        qt_sb = p_kqv.tile([P, NC, 1024], BF16, tag="qt")   # Q^T [m, t_own]
        v_sb = p_kqv.tile([P, 16, VW], BF16, tag="v")       # V_ext [s, 780]
        v_hv = v_sb.rearrange("p s (h e) -> p s h e", e=65)
        vd_view = v_hv[:, :, :, 0:64]
        nc.vector.memset(v_hv[:, :, :, 64:65], 1.0)

        es_masks = ExitStack()
        p_masks = es_masks.enter_context(tc.tile_pool(name="p_masks", bufs=1,
                                                      side="right"))
        p_e = es_masks.enter_context(tc.tile_pool(name="p_e", bufs=2,
                                                  side="right"))
        masks_sb = p_masks.tile([P, 4, TB], BF16, tag="masks")

        es_wqkv = ExitStack()
        p_wqkv = es_wqkv.enter_context(tc.tile_pool(name="p_wqkv", bufs=1,
                                                    side="right"))
        wq_sb = p_wqkv.tile([P, NC, D], E4, tag="wq")
        wqr_sb = p_wqkv.tile([P, NC, D], E4, tag="wqr")
        wk_sb = p_wqkv.tile([P, NC, D], E4, tag="wk")
        wkr_sb = p_wqkv.tile([P, NC, D], E4, tag="wkr")
        wv_sb = p_wqkv.tile([P, NC, D], E4, tag="wv")
        wvr_sb = p_wqkv.tile([P, NC, D], E4, tag="wvr")

        # ---------------- Phase A-D: LN1 + QKV over TBORD, pipelined -------
        _mark(nc, "lnq")
        es_xn1 = ExitStack()
        p_xn1 = es_xn1.enter_context(tc.tile_pool(name="p_xn1", bufs=2))
        xn1_tiles = {}

        def split8_chunk(xnb, x8_j, rx8_j, j=0):
            nc.scalar.activation(x8_j, xnb[:], AF.Copy)
            if j % 2 == 0:
                nc.vector.tensor_sub(rx8_j, xnb[:], x8_j)
            else:
                nc.gpsimd.tensor_sub(rx8_j, xnb[:], x8_j)

        def norm_units(tb):
            r, mur = stats_res[tb]
            x8_t = p_xn1.tile([P, NC, TB], E4, tag="x8", name=f"x8_{tb}")
            rx8_t = p_xn1.tile([P, NC, TB], E4, tag="rx8", name=f"rx8_{tb}")
            xn1_tiles[tb] = (x8_t, rx8_t)
            xt_t = xtr_tiles[tb]
            for j in range(NC):
                xnb = work.tile([P, TB], BF16, tag="xnb")
                ln_norm_chunk(xt_t[:, j, :], xnb[:], r, mur,
                              g1_sb, be1_sb, j)
                split8_chunk(xnb, x8_t[:, j, :], rx8_t[:, j, :], j)
                yield

        def mm3(acc, wsb, wrsb, x8, rx8, msl, n):
            """acc += (w + rw).T (x + rx), 3-term fp8 DoubleRow over j pairs."""
            first = True
            for wa, xa in ((wsb, x8), (wsb, rx8), (wrsb, x8)):
                for jp in range(NC // 2):
                    j2 = slice(2 * jp, 2 * jp + 2)
                    nc.tensor.matmul(acc[:, :n], wa[:, j2, msl], xa[:, j2, :],
                                     start=first,
                                     stop=(wa is wrsb and jp == NC // 2 - 1),
                                     perf_mode=MM)
                    first = False

        def mm3t(acc, x8, rx8, ssl, wsb, wrsb, fsl, n):
            """acc += (x + rx).T (w + rw): x stationary variant."""
            first = True
            for xa, wa in ((x8, wsb), (x8, wrsb), (rx8, wsb)):
                for jp in range(NC // 2):
                    j2 = slice(2 * jp, 2 * jp + 2)
                    nc.tensor.matmul(acc[:, :n], xa[:, j2, ssl], wa[:, j2, fsl],
                                     start=first,
                                     stop=(xa is rx8 and jp == NC // 2 - 1),
                                     perf_mode=MM)
                    first = False

        def qkv_units(tb, want_q):
            tsl = slice(tb * TB, (tb + 1) * TB)
            x8_t, rx8_t = xn1_tiles[tb]
            for mt in range(NC):
                msl = slice(mt * P, (mt + 1) * P)
                acc = ps.tile([P, TB], F32, tag="acc", bufs=2, name="ka")
                mm3(acc, wk_sb, wkr_sb, x8_t, rx8_t, msl, TB)
                nc.vector.tensor_scalar_mul(kt_sb[:, mt, tsl], acc[:], DS)
                yield
            for st in range(4):
                ssl = slice(st * P, (st + 1) * P)
                for half, fsl, w in ((0, slice(0, TB), TB),
                                     (1, slice(TB, D), D - TB)):
                    acc = ps.tile([P, TB], F32, tag="acc", bufs=2, name="va")
                    mm3t(acc, x8_t, rx8_t, ssl, wv_sb, wvr_sb, fsl, w)
                    src = acc[:, :w].rearrange("p (h e) -> p h e", e=64)
                    h0 = half * 8
                    dst = vd_view[:, tb * 4 + st, h0:h0 + w // 64, :]
                    nc.vector.tensor_scalar_mul(dst, src, DS)
                    yield
            if want_q:
                for mt in range(NC):
                    msl = slice(mt * P, (mt + 1) * P)
                    acc = ps.tile([P, TB], F32, tag="acc", bufs=2, name="qa")
                    mm3(acc, wq_sb, wqr_sb, x8_t, rx8_t, msl, TB)
                    nc.vector.tensor_scalar_mul(qt_sb[:, mt, tsl], acc[:], DS)
                    yield

        es_xtr = ExitStack()
        p_xtr = es_xtr.enter_context(tc.tile_pool(name="p_xtr", bufs=2))
        xtr_tiles = {}
        stats_res = {}

        def load_xt(tb, step=2):
            tsl = slice(tb * TB, (tb + 1) * TB)
            xt_t = p_xtr.tile([P, NC, TB], F32R, tag="xtr", name=f"xt_{tb}")
            xtr_tiles[tb] = xt_t
            for j0 in range(0, NC, step):
                nc.sync.dma_start(xt_t[:, j0:j0 + step, :],
                                  xt_r[:, j0:j0 + step, tsl])

        # A: tb0 stats
        _early_const_dmas()
        load_xt(0, step=1)
        for sb, d in ((wk_sb, wk8_d), (wkr_sb, wk8r_d), (wv_sb, wv8_d),
                      (wvr_sb, wv8r_d), (wq_sb, wq8_d), (wqr_sb, wq8r_d)):
            nc.sync.dma_start(sb[:], d.rearrange("(j p) m -> p j m", p=P))
        stats_res[0] = ln_stats(xtr_tiles[0][:], sq_pool=True)
        # B: tb2 stats; norm0 + qkv0
        load_xt(2)
        _late_const_dmas()
        stats_res[2] = ln_stats(xtr_tiles[2][:], sq_pool=True)
        for _ in norm_units(0):
            pass
        for _ in qkv_units(0, want_q=True):
            pass
        # C: tb1 stats; norm2 + qkv2
        load_xt(1)
        stats_res[1] = ln_stats(xtr_tiles[1][:], sq_pool=True)
        for _ in norm_units(2):
            pass
        for _ in qkv_units(2, want_q=False):
            pass
        # D: tb3 stats; norm1
        load_xt(3)
        stats_res[3] = ln_stats(xtr_tiles[3][:], sq_pool=True)
        for _ in norm_units(1):
            pass

        # ---------------- attention machinery ----------------

        def attn_units(sl_i):
            qsl = slice(sl_i * TB, (sl_i + 1) * TB)
            chunks = SLOT_CHUNKS[sl_i]
            last_ci = len(chunks) - 1
            for mt in range(NC):
                yt2 = [ps.tile([65, TB], F32, tag="yt", bufs=2,
                               name=f"yt{ph}") for ph in range(2)]

                def pv(ci, e_sb):
                    ch = chunks[ci]
                    for ph in range(2):
                        h = 2 * mt + ph
                        nc.tensor.matmul(
                            yt2[ph][:], v_sb[:, ch, h * 65:(h + 1) * 65],
                            e_sb[:, ph * TB:(ph + 1) * TB],
                            start=(ci == 0), stop=(ci == last_ci))

                pend = None
                for ci, ch in enumerate(chunks):
                    sb_idx = (0 if sl_i == 0 else 8) + ci
                    st2 = ps.tile([P, 2 * TB], F32, tag="st", bufs=2)
                    for ph in range(2):
                        o = ph * 64
                        nc.tensor.matmul(
                            st2[:, ph * TB:(ph + 1) * TB],
                            kt_sb[o:o + 64, mt, ch * P:(ch + 1) * P],
                            qt_sb[o:o + 64, mt, qsl], start=True, stop=True)
                    e_sb = p_e.tile([P, 2 * TB], BF16, tag="e")
                    nc.scalar.activation(
                        e_sb[:], st2[:], AF.Exp,
                        bias=bias_sb[:, sb_idx:sb_idx + 1],
                        scale=scale_sb[:, sb_idx:sb_idx + 1])
                    di = ch - 4 * sl_i
                    if 0 <= di < 4:
                        for ph in range(2):
                            nc.vector.tensor_mul(
                                e_sb[:, ph * TB:(ph + 1) * TB],
                                e_sb[:, ph * TB:(ph + 1) * TB],
                                masks_sb[:, di, :])
                    if pend is not None:
                        pv(*pend)
                    pend = (ci, e_sb)
                    yield
                pv(*pend)
                for ph in range(2):
                    o = ph * 64
                    rc = work.tile([1, TB], F32R, tag="rc")
                    nc.vector.reciprocal(rc[:], yt2[ph][64:65, :])
                    bc = ps.tile([64, TB], F32, tag="acc", bufs=2, name="bc")
                    nc.tensor.matmul(bc[:], onesbc_sb[0:1, 0:64], rc[:],
                                     start=True, stop=True)
                    nc.vector.tensor_copy(yt_all[o:o + 64, mt, :],
                                          yt2[ph][0:64, :])
                    nc.vector.tensor_mul(yt_all[o:o + 64, mt, :],
                                         yt_all[o:o + 64, mt, :], bc[:])
                yield

        def wo_ln2_units(sl_i):
            qsl = slice(sl_i * TB, (sl_i + 1) * TB)
            x28_sb, rx28_sb = S["xn2_sb"]
            wo_sb = S["wo_sb"]
            for ct in range(NC):
                ao = ps.tile([P, TB], F32, tag="acc", bufs=2, name="ao")
                for mc in range(NC):
                    nc.tensor.matmul(ao[:], wo_sb[:, mc, ct * P:(ct + 1) * P],
                                     yt_all[:, mc, :],
                                     start=(mc == 0), stop=(mc == NC - 1))
                nc.vector.tensor_add(xp_sb[:, ct, qsl],
                                     xp_sb[:, ct, qsl], ao[:])
                yield
            r2, mur2 = ln_stats(xp_sb[:, :, qsl])
            yield
            for j in range(NC):
                xnb = work.tile([P, TB], BF16, tag="xnb")
                ln_norm_chunk(xp_sb[:, j, qsl], xnb[:], r2, mur2,
                              g2_sb, be2_sb, j)
                split8_chunk(xnb, x28_sb[:, j, qsl], rx28_sb[:, j, qsl], j)
                yield

        # ---------------- MLP machinery ----------------
        w18_r = w18_d.rearrange("(j p) f -> p j f", p=P)
        w18r_r = w18r_d.rearrange("(j p) f -> p j f", p=P)
        w28_r = w28_d.rearrange("(f p) c -> p f c", p=P)
        w28r_r = w28r_d.rearrange("(f p) c -> p f c", p=P)
        outt_r = outt_d.rearrange("(j p) t -> p j t", p=P)

        def w1_units(sl_i):
            tsl = slice(sl_i * TB, (sl_i + 1) * TB)
            (h8_sb, rh8_sb), p_wmlp = S["h1_sb"], S["p_wmlp"]
            x28_sb, rx28_sb = S["xn2_sb"]

            def w1_dma(ft2):
                fs = slice(2 * ft2 * P, (2 * ft2 + 2) * P)
                w1_t = p_wmlp.tile([P, NC, 2 * P], E4, tag="w1", bufs=3,
                                   name=f"w1_{sl_i}_{ft2}")
                w1r_t = p_wmlp.tile([P, NC, 2 * P], E4, tag="w1r", bufs=3,
                                    name=f"w1r_{sl_i}_{ft2}")
                nc.sync.dma_start(w1_t[:], w18_r[:, :, fs])
                nc.sync.dma_start(w1r_t[:], w18r_r[:, :, fs])
                return w1_t, w1r_t
            nxt = w1_dma(0)
            for ft2 in range(NF // 2):
                w1_t, w1r_t = nxt
                if ft2 + 1 < NF // 2:
                    nxt = w1_dma(ft2 + 1)
                for sub in range(2):
                    ft = 2 * ft2 + sub
                    msl = slice(sub * P, (sub + 1) * P)
                    hp = ps.tile([P, TB], F32, tag="acc", bufs=2, name="hp")
                    mm3(hp, w1_t, w1r_t, x28_sb[:, :, tsl],
                        rx28_sb[:, :, tsl], msl, TB)
                    hb = work.tile([P, TB], F32R, tag="nrm1", name="hb")
                    nc.vector.tensor_scalar(hb[:], hp[:], DS,
                                            b1_sb[:, ft:ft + 1],
                                            ALU.mult, ALU.add)
                    hbf = work.tile([P, TB], BF16, tag="xnb", name="hbf")
                    nc.vector.tensor_scalar_max(hbf[:], hb[:], 0.0)
                    nc.vector.tensor_copy(h8_sb[:, ft, tsl], hbf[:])
                    nc.gpsimd.tensor_sub(rh8_sb[:, ft, tsl], hbf[:],
                                         h8_sb[:, ft, tsl])
                    yield

        w2_tiles = {}

        def w2_dma_units():
            p_w2 = S["p_w2"]
            for ft2 in range(NF // 2):
                fs = slice(2 * ft2, 2 * ft2 + 2)
                w2_t = p_w2.tile([P, 2, D], E4, tag=f"w2_{ft2}", bufs=1,
                                 name=f"w2_{ft2}")
                w2r_t = p_w2.tile([P, 2, D], E4, tag=f"w2r_{ft2}", bufs=1,
                                  name=f"w2r_{ft2}")
                nc.sync.dma_start(w2_t[:], w28_r[:, fs, :])
                nc.sync.dma_start(w2r_t[:], w28r_r[:, fs, :])
                w2_tiles[ft2] = (w2_t, w2r_t)
                yield

        def w2_units(g):
            (h8_sb, rh8_sb) = S["h1_sb"]
            p_out = S["p_out"]
            o2s = {}
            for ci, (t, bu) in enumerate((("acc", 2), ("yt", 2))):
                for tb in range(2):
                    o2s[(ci, tb)] = ps.tile([P, TB], F32, tag=t, bufs=bu,
                                            name=f"o2_{g}_{ci}_{tb}")
            stp = ps.tile([P, 2 * TB], F32, tag="st", bufs=2,
                          name=f"o2st_{g}")
            o2s[(2, 0)] = stp[:, 0:TB]
            o2s[(2, 1)] = stp[:, TB:2 * TB]

            for ft2 in range(NF // 2):
                w2_t, w2r_t = w2_tiles[ft2]
                for ci in range(3):
                    ct = g * 3 + ci
                    csl = slice(ct * P, (ct + 1) * P)
                    for tb in range(2):
                        hsl = slice(tb * TB, (tb + 1) * TB)
                        for wa, ha in ((w2_t, h8_sb), (w2_t, rh8_sb),
                                       (w2r_t, h8_sb)):
                            nc.tensor.matmul(
                                o2s[(ci, tb)][:], wa[:, :, csl],
                                ha[:, 2 * ft2:2 * ft2 + 2, hsl],
                                start=(ft2 == 0 and wa is w2_t
                                       and ha is h8_sb),
                                stop=(ft2 == NF // 2 - 1 and wa is w2r_t),
                                perf_mode=MM)
                yield
            for tb in range(2):
                tsl = slice(tb * TB, (tb + 1) * TB)
                for ci in range(3):
                    ct = g * 3 + ci
                    xpb = work.tile([P, TB], F32R, tag="nrm1", name="xpb")
                    nc.vector.tensor_scalar(xpb[:], xp_sb[:, ct, tsl],
                                            b2_sb[:, ct:ct + 1], None,
                                            ALU.add)
                    ot = p_out.tile([P, TB], F32, tag="ot")
                    nc.vector.scalar_tensor_tensor(
                        ot[:], o2s[(ci, tb)][:], DS, xpb[:],
                        ALU.mult, ALU.add)
                    nc.sync.dma_start(outt_r[:, ct, tsl], ot[:])
                    yield

        # ---------------- Phase E: slot0 attention woven with rest --------
        _mark(nc, "attn0")

        def e_fill():
            for u in norm_units(3):
                yield u
            es_xtr.close()
            for u in qkv_units(1, want_q=True):
                yield u
            for u in qkv_units(3, want_q=False):
                yield u
            es_xn1.close()
            es_wqkv.close()
            p_h1 = ctx.enter_context(tc.tile_pool(name="p_h1", bufs=1))
            S["h1_sb"] = (
                p_h1.tile([P, NF, 1024], E4, tag="h8", name="h8"),
                p_h1.tile([P, NF, 1024], E4, tag="rh8", name="rh8"))
            S["p_wmlp"] = ctx.enter_context(
                tc.tile_pool(name="p_wmlp", bufs=2))
            S["p_wo"] = es_wo = ExitStack()
            p_wo = es_wo.enter_context(tc.tile_pool(name="p_wo", bufs=1))
            S["wo_sb"] = wo_sb = p_wo.tile([P, NC, D], BF16, tag="wo", name="wo")
            nc.sync.dma_start(wo_sb[:],
                              wo_d.rearrange("(j p) m -> p j m", p=P))
            nc.sync.dma_start(xp_sb[:], xt_r[:, :, 0:1024])
            yield

        _weave(attn_units(0), e_fill(), fill_per=0.7)

        # ---------------- Phase F: slot0 wo + LN2 ----------------
        _mark(nc, "wo0")
        S["p_xn2"] = es_xn2 = ExitStack()
        p_xn2 = es_xn2.enter_context(tc.tile_pool(name="p_xn2", bufs=1))
        S["xn2_sb"] = (
            p_xn2.tile([P, NC, 1024], E4, tag="x28", name="x28"),
            p_xn2.tile([P, NC, 1024], E4, tag="rx28", name="rx28"))
        for _ in wo_ln2_units(0):
            pass

        # ---------------- Phase G: slot1 attention woven with W1(slot0) ---
        _mark(nc, "attn1")
        from itertools import islice
        w1g0 = w1_units(0)
        _weave(attn_units(1), islice(w1g0, 16), fill_per=0.17)

        # ---------------- Phase H: slot1 wo + LN2 woven with W1 rest ------
        _mark(nc, "wo1")
        _weave(wo_ln2_units(1), w1g0, fill_per=1.0)
        es_masks.close()
        es_kqv.close()
        _mark(nc, "mlp")
        S["p_w2"] = ctx.enter_context(
            tc.tile_pool(name="p_w2", bufs=1, side="right"))
        _weave(w1_units(1), w2_dma_units(), fill_per=1.0)
        S["p_xn2"].close()
        S["p_wo"].close()
        S["p_out"] = ctx.enter_context(tc.tile_pool(name="p_out", bufs=3))
        for _ in w2_units(0):
            pass
        for _ in w2_units(1):
            pass

    nc.compile()
    return nc


def _host_inputs(X, w_q, w_k, w_v, w_o, W1, b1, W2, b2, g1, be1, g2, be2):
    """Build the 8 per-core input dicts."""
    f32 = np.float32
    import ml_dtypes
    bf16 = ml_dtypes.bfloat16
    e4 = ml_dtypes.float8_e4m3

    def split8(w):
        ws = np.asarray(w, f32) * WS
        a = ws.astype(e4)
        r = (ws - a.astype(f32)).astype(e4)
        return np.ascontiguousarray(a), np.ascontiguousarray(r)

    wq8, wq8r = split8(np.asarray(w_q, f32).reshape(D, D).T)
    wk8, wk8r = split8(np.asarray(w_k, f32).reshape(D, D).T)
    wv8, wv8r = split8(np.asarray(w_v, f32).reshape(D, D).T)
    wo = np.ascontiguousarray(np.asarray(w_o, f32).astype(bf16))
    onesbc = np.ones((P, P), f32)
    # 4 canonical self-diagonal masks: mask[k][s, t] = (128k + s <= t)
    masks = np.zeros((4, P, TB), bf16)
    ar_s = np.arange(P)[:, None]
    ar_t = np.arange(TB)[None, :]
    for k in range(4):
        masks[k] = (128 * k + ar_s <= ar_t).astype(bf16)
    w18, w18r = split8(np.asarray(W1, f32).T)
    w28, w28r = split8(np.asarray(W2, f32).T)

    # per-role exp scale/bias: 24 = 8 (slot0) + 16 (slot1) chunk positions
    sc = {}
    bi = {}
    for role in range(2):
        order = ROLE_ORDER[role]
        s = np.full((24,), 0.125, f32)
        b = np.zeros((24,), f32)
        for sl_i in range(2):
            own_blk = order[sl_i]
            for ci, ch in enumerate(SLOT_CHUNKS[sl_i]):
                idx = (0 if sl_i == 0 else 8) + ci
                pos = ch // 4           # permuted 512-block of this s-chunk
                blk = order[pos]
                if pos == sl_i or blk < own_blk:
                    pass                # diagonal (tri-masked) or past: live
                else:
                    s[idx] = 0.0        # future: dead
                    b[idx] = DEAD
        sc[role] = np.broadcast_to(s, (P, 24)).copy()
        bi[role] = np.broadcast_to(b, (P, 24)).copy()

    shared = dict(wq8=wq8, wq8r=wq8r, wk8=wk8, wk8r=wk8r, wv8=wv8,
                  wv8r=wv8r, wo=wo, w18=w18, w18r=w18r, w28=w28, w28r=w28r,
                  onesbc=onesbc, masks=masks,
                  g1v=np.asarray(g1, f32), be1v=np.asarray(be1, f32),
                  g2v=np.asarray(g2, f32), be2v=np.asarray(be2, f32),
                  b1v=np.asarray(b1, f32), b2v=np.asarray(b2, f32))

    in_maps = []
    for core in range(8):
        role, b_idx = core // 4, core % 4
        order = ROLE_ORDER[role]
        xb = np.asarray(X[b_idx], f32)          # [T, D]
        xperm = np.concatenate([xb[o * TB:(o + 1) * TB] for o in order], axis=0)
        xt = np.ascontiguousarray(xperm.T)      # [D, T]
        m = dict(shared)
        m["xt"] = xt
        m["scalein"] = sc[role]
        m["biasin"] = bi[role]
        in_maps.append(m)
    return in_maps


def _assemble(results, dtype):
    out = np.empty((B, T, D), dtype)
    for core in range(8):
        role, b_idx = core // 4, core % 4
        order = ROLE_ORDER[role]
        ot = results[core]["outt"]              # [D, 1024]
        for sl_i in range(2):
            blk = order[sl_i]
            out[b_idx, blk * TB:(blk + 1) * TB] = \
                ot[:, sl_i * TB:(sl_i + 1) * TB].T
    return out


def kernel(X, w_q, w_k, w_v, w_o, W1, b1, W2, b2, g1, be1, g2, be2,
           _want_results=False, _trace=False):
    if "nc" not in _cached:
        _cached["nc"] = _build_nc()
    nc = _cached["nc"]
    in_maps = _host_inputs(X, w_q, w_k, w_v, w_o, W1, b1, W2, b2,
                           g1, be1, g2, be2)
    res = run_bass_kernel_spmd(nc, in_maps, core_ids=list(range(8)),
                               trace=_trace)
    out = _assemble(res.results, np.asarray(X).dtype)
    if _want_results:
        return out, res
    return out
